# revision 2
# baseline (speedup 1.0000x reference)
"""Trainium2 Bass kernel for CNN-encoder + attention-LSTM captioner + vocab FC.

Sharding: pure data-parallel over batch (16 images -> 8 cores x 2 images).
All weights replicated; no collectives. Host slices inputs / concatenates outputs.

Layout conventions (per core, B=2 local images, T=32 steps):
  - tokens are indexed p = t*2 + b  (t-major) so each LSTM step reads a
    contiguous partition pair from the batched precompute.
  - recurrent state h is kept transposed ([HID, 2] chunks) in outsT so it can
    feed the next step's matmul lhsT directly and the final FC lhsT.
"""

import os
import numpy as np

os.environ.setdefault("MYCRO_LOCAL_CACHE", "1")

HID = 640
VOCAB = 10000
T = 32
BL = 2            # local batch per core
NTOK = T * BL     # 64
NCORES = 8

F32 = None  # set lazily (mybir.dt.float32)


class _PhaseExit(Exception):
    def __init__(self, tc):
        self.tc = tc

_NC_CACHE = {}


def _gate_perm():
    # reference gate order [i, f, g, o] -> kernel order [i, f, o, g]
    return np.concatenate([
        np.arange(0, 1280),          # i, f
        np.arange(1920, 2560),       # o
        np.arange(1280, 1920),       # g
    ])


def build_bass(upto=None):
    import os
    upto = upto or os.environ.get("KERNEL_UPTO", "all")
    import concourse.bass as bass
    from concourse import bacc
    import concourse.tile_sem_assignment as tsa
    # Cap HWDGE sem lanes so pool-transition fan-ins stay under the
    # per-instruction sync-wait slot limits in walrus codegen.
    tsa.NUM_HWDGE_SEMS = 4
    import concourse.mybir as mybir
    import concourse.tile as tile
    from concourse.masks import make_identity

    f32 = mybir.dt.float32
    i32 = mybir.dt.int32
    AF = mybir.ActivationFunctionType
    ALU = mybir.AluOpType
    AX = mybir.AxisListType

    nc = bacc.Bacc(None)
    bf16 = mybir.dt.bfloat16

    def mm(out, lhsT, rhs, **kw):
        nc.tensor.matmul(out=out, lhsT=lhsT, rhs=rhs, **kw)

    # ---------------- DRAM parameters ----------------
    img_d = nc.declare_dram_parameter("img", [BL, 27, 224 * 224], bf16, isOutput=False)
    caps_d = nc.declare_dram_parameter("caps", [NTOK, 1], i32, isOutput=False)
    w1b_d = nc.declare_dram_parameter("w1b", [27, 64], bf16, isOutput=False)
    cb1_d = nc.declare_dram_parameter("cb1t", [64, 1], f32, isOutput=False)
    cb2_d = nc.declare_dram_parameter("cb2t", [128, 1], f32, isOutput=False)
    w2t9_d = nc.declare_dram_parameter("w2t9", [9, 64, 128], bf16, isOutput=False)
    w3t9_d = nc.declare_dram_parameter("w3t9", [9, 128, 256], bf16, isOutput=False)
    w4t9_d = nc.declare_dram_parameter("w4t9", [9, 2, 128, 512], bf16, isOutput=False)
    cb3_d = nc.declare_dram_parameter("cb3t", [128, 2], f32, isOutput=False)
    cb4_d = nc.declare_dram_parameter("cb4t", [128, 4], f32, isOutput=False)
    encw_d = nc.declare_dram_parameter("encwt", [4, 128, HID], f32, isOutput=False)
    encb_d = nc.declare_dram_parameter("encbt", [128, 5], f32, isOutput=False)
    emb_d = nc.declare_dram_parameter("emb", [VOCAB, HID], bf16, isOutput=False)
    attnw_d = nc.declare_dram_parameter("attnwt", [10, 128, HID], bf16, isOutput=False)
    attnb_d = nc.declare_dram_parameter("attnb", [1, HID], bf16, isOutput=False)
    wih_d = nc.declare_dram_parameter("wiht", [10, 128, 4 * HID], bf16, isOutput=False)
    whh_d = nc.declare_dram_parameter("whht", [5, 128, 4 * HID], bf16, isOutput=False)
    bgate_d = nc.declare_dram_parameter("bgate", [1, 4 * HID], bf16, isOutput=False)
    fcw_d = nc.declare_dram_parameter("fcwt", [5, 128, VOCAB], bf16, isOutput=False)
    fcb_d = nc.declare_dram_parameter("fcb", [1, VOCAB], bf16, isOutput=False)
    bsel_d = nc.declare_dram_parameter("bsel", [BL, NTOK], f32, isOutput=False)
    logits_d = nc.declare_dram_parameter("logits", [BL, T, VOCAB], f32, isOutput=True)

    try:
      with tile.TileContext(nc) as tc:
        # ---------------- persistent constants ----------------
        cpool = tc.alloc_tile_pool(name="const", bufs=1)
        # pool for all DMA-written tiles: never released mid-kernel so that
        # SBUF zone reuse never makes compute ops wait on DMA queue sems
        dmapool = tc.alloc_tile_pool(name="dmat", bufs=1)
        ident = cpool.tile([128, 128], f32)
        make_identity(nc, ident[:, :])
        identb = cpool.tile([128, 128], bf16)
        make_identity(nc, identb[:, :])
        ones64 = cpool.tile([1, 64], bf16)
        nc.gpsimd.memset(ones64[:, :], 1.0)
        bsel_sb = dmapool.tile([BL, NTOK], f32)
        nc.sync.dma_start(out=bsel_sb[:, :], in_=bsel_d[:, :])
        feat_sb = cpool.tile([128, 4, BL], f32)   # feat.T, K-chunked [128,4] per img

        w1b_sb = dmapool.tile([27, 64], bf16)
        nc.sync.dma_start(out=w1b_sb[:, :], in_=w1b_d[:, :])
        cb1_sb = dmapool.tile([64, 1], f32)
        nc.sync.dma_start(out=cb1_sb[:, :], in_=cb1_d[:, :])
        cb2_sb = dmapool.tile([128, 1], f32)
        nc.sync.dma_start(out=cb2_sb[:, :], in_=cb2_d[:, :])
        w2_sb = dmapool.tile([64, 9, 128], bf16)
        nc.sync.dma_start(out=w2_sb[:, :, :], in_=w2t9_d[:, :, :].rearrange("t p o -> p t o"))
        w3_sb = dmapool.tile([128, 9, 256], bf16)
        nc.sync.dma_start(out=w3_sb[:, :, :], in_=w3t9_d[:, :, :].rearrange("t p o -> p t o"))
        cb3_sb = dmapool.tile([128, 2], f32)
        nc.sync.dma_start(out=cb3_sb[:, :], in_=cb3_d[:, :])
        cb4_sb = dmapool.tile([128, 4], f32)
        nc.sync.dma_start(out=cb4_sb[:, :], in_=cb4_d[:, :])

        # ---------------- conv tower, per image ----------------
        for im in range(BL):
            ipool = tc.alloc_tile_pool(name=f"img{im}", bufs=1)
            # padded pool1 output (input of conv2)
            x2_pad = ipool.tile([64, 114, 114], bf16)
            nc.vector.memset(x2_pad[:, :, :], 0.0)

            # ---- conv1 (3->64) via im2col K=27, flat reads from padded DRAM ----
            c1pool = tc.alloc_tile_pool(name=f"c1_{im}", bufs=2)
            c1psum = tc.alloc_tile_pool(name=f"c1p_{im}", bufs=3, space="PSUM")
            R = 16
            for ch in range(224 // R):
                Y = R * ch
                rh = dmapool.tile([27, R * 224], bf16, tag="rh", bufs=2)
                nc.sync.dma_start(out=rh[:, :],
                                  in_=img_d[im, :, Y * 224:(Y + R) * 224])
                rhv = rh.rearrange("p (r x) -> p r x", x=224)
                # 4 psum tiles of 4 rows (2 subtiles x 2 rows)
                for q in range(4):
                    ps = c1psum.tile([64, 2, 448], f32, padded_shape=[64, 2, 512], tag="ps")
                    for s in range(2):
                        r0 = q * 4 + s * 2
                        mm(
                            out=ps[:, s, :],
                            lhsT=w1b_sb[:, :],
                            rhs=rhv[:, r0:r0 + 2, :],
                            start=True, stop=True,
                        )
                    a1 = c1pool.tile([64, 4, 224], bf16, tag="a1")
                    nc.scalar.activation(
                        a1.rearrange("p (a b) x -> p a b x", b=2),
                        ps.rearrange("p a (b x) -> p a b x", b=2),
                        AF.Relu, bias=cb1_sb[:, 0:1])
                    t1 = c1pool.tile([64, 4, 112], bf16, tag="t1")
                    nc.vector.tensor_tensor(
                        out=t1[:, :, :],
                        in0=a1[:, :, 0:224:2], in1=a1[:, :, 1:224:2],
                        op=ALU.max,
                    )
                    # pool rows: out rows (16ch+4q)/2 .. +2
                    oy = (R * ch + 4 * q) // 2
                    nc.vector.tensor_tensor(
                        out=x2_pad[:, oy + 1:oy + 3, 1:113],
                        in0=t1[:, 0:4:2, :], in1=t1[:, 1:4:2, :],
                        op=ALU.max,
                    )
            c1psum.release()
            c1pool.release()

            # ---- conv2 (64->128) 9 taps K=64 (tap4 K=65 adds bias), pool -> x3_pad ----
            x3_pad = ipool.tile([128, 58, 58], bf16)
            nc.vector.memset(x3_pad[:, :, :], 0.0)
            c2psum = tc.alloc_tile_pool(name=f"c2p_{im}", bufs=3, space="PSUM")
            c2pool = tc.alloc_tile_pool(name=f"c2_{im}", bufs=2)
            for tl in range(14):  # 8 output rows per tile
                ps = c2psum.tile([128, 2, 448], f32, padded_shape=[128, 2, 512], tag="ps")
                for s in range(2):
                    y0 = tl * 8 + s * 4
                    for ky in range(3):
                        for kx in range(3):
                            tap = ky * 3 + kx
                            mm(
                                out=ps[:, s, :], lhsT=w2_sb[:, tap, :],
                                rhs=x2_pad[:, y0 + ky:y0 + ky + 4, kx:kx + 112],
                                start=(tap == 0), stop=(tap == 8),
                            )
                a2 = c2pool.tile([128, 8, 112], bf16, tag="a2")
                nc.scalar.activation(
                    a2.rearrange("p (a y) x -> p a y x", a=2),
                    ps.rearrange("p a (y x) -> p a y x", x=112),
                    AF.Relu, bias=cb2_sb[:, 0:1])
                t2 = c2pool.tile([128, 8, 56], bf16, tag="t2")
                nc.vector.tensor_tensor(
                    out=t2[:, :, :], in0=a2[:, :, 0:112:2], in1=a2[:, :, 1:112:2],
                    op=ALU.max,
                )
                t2b = c2pool.tile([128, 4, 56], bf16, tag="t2b")
                nc.vector.tensor_tensor(
                    out=t2b[:, :, :], in0=t2[:, 0:8:2, :], in1=t2[:, 1:8:2, :],
                    op=ALU.max,
                )
                nc.vector.tensor_copy(
                    out=x3_pad[:, tl * 4 + 1:tl * 4 + 5, 1:57],
                    in_=t2b[:, :, :],
                )
            c2psum.release()
            c2pool.release()

            # ---- conv3 (128->256) K=128, bias via ACT evict, pool -> x4_pad ----
            x4_pad = ipool.tile([128, 2, 30, 30], bf16)
            nc.vector.memset(x4_pad[:, :, :, :], 0.0)
            c3psum = tc.alloc_tile_pool(name=f"c3p_{im}", bufs=3, space="PSUM")
            c3pool = tc.alloc_tile_pool(name=f"c3_{im}", bufs=2)
            for m in range(2):
                for tl in range(7):  # 8 output rows per tile
                    ps = c3psum.tile([128, 448], f32, padded_shape=[128, 512], tag="ps")
                    y0 = tl * 8
                    for ky in range(3):
                        for kx in range(3):
                            tap = ky * 3 + kx
                            rhs = x3_pad[:, y0 + ky:y0 + ky + 8, kx:kx + 56]
                            mm(
                                out=ps[:, :],
                                lhsT=w3_sb[:, tap, 128 * m:128 * (m + 1)],
                                rhs=rhs,
                                start=(tap == 0), stop=(tap == 8),
                            )
                    a3 = c3pool.tile([128, 8, 56], bf16, tag="a3")
                    nc.scalar.activation(
                        a3[:, :, :],
                        ps.rearrange("p (y x) -> p y x", x=56),
                        AF.Relu, bias=cb3_sb[:, m:m + 1])
                    t3 = c3pool.tile([128, 8, 28], bf16, tag="t3")
                    nc.vector.tensor_tensor(
                        out=t3[:, :, :], in0=a3[:, :, 0:56:2], in1=a3[:, :, 1:56:2],
                        op=ALU.max,
                    )
                    nc.vector.tensor_tensor(
                        out=x4_pad[:, m, tl * 4 + 1:tl * 4 + 5, 1:29],
                        in0=t3[:, 0:8:2, :], in1=t3[:, 1:8:2, :],
                        op=ALU.max,
                    )
            c3psum.release()
            c3pool.release()

            # ---- conv4 (256->512) K=256 (2 chunks), no pool; mean via accum_out ----
            c4psum = tc.alloc_tile_pool(name=f"c4p_{im}", bufs=3, space="PSUM")
            c4pool = tc.alloc_tile_pool(name=f"c4_{im}", bufs=2)
            msum = ipool.tile([128, 4, 2], f32)
            for m in range(4):
                w4m = dmapool.tile([128, 2, 9, 128], bf16, tag="w4m", bufs=2)
                for k2 in range(2):
                    nc.sync.dma_start(
                        out=w4m[:, k2, :, :],
                        in_=w4t9_d[:, k2, :, 128 * m:128 * (m + 1)].rearrange(
                            "t p o -> p t o"),
                    )
                ps = c4psum.tile([128, 2, 392], f32, padded_shape=[128, 2, 512], tag="ps")
                for s in range(2):
                    y0 = s * 14
                    first = True
                    for ky in range(3):
                        for kx in range(3):
                            tap = ky * 3 + kx
                            for k2 in range(2):
                                rhs = x4_pad[:, k2, y0 + ky:y0 + ky + 14, kx:kx + 28]
                                mm(
                                    out=ps[:, s, :],
                                    lhsT=w4m[:, k2, tap, :],
                                    rhs=rhs,
                                    start=first, stop=(tap == 8 and k2 == 1),
                                )
                                first = False
                a4 = c4pool.tile([128, 2, 392], bf16, tag="a4")
                for s in range(2):
                    nc.scalar.activation(a4[:, s, :], ps[:, s, :], AF.Relu,
                                         bias=cb4_sb[:, m:m + 1],
                                         accum_out=msum[:, m, s:s + 1])
            c4psum.release()
            c4pool.release()
            # feat.T[:, m] = (msum[:,m,0] + msum[:,m,1]) / 784
            tmpf = ipool.tile([128, 4], f32)
            nc.vector.tensor_tensor(out=tmpf[:, :], in0=msum[:, :, 0], in1=msum[:, :, 1],
                                    op=ALU.add)
            nc.vector.tensor_scalar_mul(feat_sb[:, :, im], tmpf[:, :], 1.0 / 784.0)
            ipool.release()

        if upto == "conv":
            raise _PhaseExit(tc)

        # ---------------- encoder linear: memory.T = enc_w @ feat.T + enc_b ----------------
        spool = tc.alloc_tile_pool(name="seq", bufs=1)
        scpool = tc.alloc_tile_pool(name="scratch", bufs=1)
        encw_sb = dmapool.tile([128, 4, HID], f32)
        nc.sync.dma_start(out=encw_sb[:, :, :], in_=encw_d[:, :, :].rearrange("k p o -> p k o"))
        encb_sb = dmapool.tile([128, 5], f32)
        nc.sync.dma_start(out=encb_sb[:, :], in_=encb_d[:, :])

        p1psum = tc.alloc_tile_pool(name="p1ps", bufs=1, space="PSUM")
        memT_ps = p1psum.tile([128, 5, BL], f32)
        for m in range(5):
            for k in range(4):
                nc.tensor.matmul(
                    out=memT_ps[:, m, :],
                    lhsT=encw_sb[:, k, 128 * m:128 * (m + 1)],
                    rhs=feat_sb[:, k, :],
                    start=(k == 0), stop=(k == 3),
                )
        memT_sb = spool.tile([128, 5, BL], f32)
        for m in range(5):
            nc.vector.tensor_scalar_add(memT_sb[:, m, :], memT_ps[:, m, :],
                                        encb_sb[:, m:m + 1])
        # memory non-transposed [2, 640]
        mem_ps = p1psum.tile([BL, HID], f32)
        for m in range(5):
            nc.tensor.transpose(out=mem_ps[:, 128 * m:128 * (m + 1)],
                                in_=memT_sb[:, m, :], identity=ident[:, :])
        mem_sb = scpool.tile([BL, HID], f32)
        nc.scalar.copy(mem_sb[:, :], mem_ps[:, :])

        # memory broadcast to all tokens [64, 640] via bsel matmul
        mexp_ps = p1psum.tile([NTOK, HID], f32)
        for n in range(2):
            sl = slice(512 * n, min(HID, 512 * (n + 1)))
            nc.tensor.matmul(out=mexp_ps[:, sl], lhsT=bsel_sb[:, :], rhs=mem_sb[:, sl],
                             start=True, stop=True)
        mexp_sb = scpool.tile([NTOK, HID], f32)
        nc.scalar.copy(mexp_sb[:, :], mexp_ps[:, :])
        p1psum.release()
        p1bpsum = tc.alloc_tile_pool(name="p1bps", bufs=1, space="PSUM")

        # ---------------- embeddings gather + fusedT ----------------
        idx_sb = dmapool.tile([NTOK, 1], i32)
        nc.sync.dma_start(out=idx_sb[:, :], in_=caps_d[:, :])
        e_sb = dmapool.tile([NTOK, HID], bf16)
        nc.gpsimd.indirect_dma_start(
            out=e_sb[:, :], out_offset=None,
            in_=emb_d[:, :],
            in_offset=bass.IndirectOffsetOnAxis(ap=idx_sb[:, :1], axis=0),
        )
        # fusedT [128, 10, 64]: chunks 0-4 = e.T ; 5-9 = memory.T broadcast
        fusedT_pse = p1bpsum.tile([128, 5, NTOK], bf16)
        for k in range(5):
            nc.tensor.transpose(out=fusedT_pse[:, k, :],
                                in_=e_sb[:, 128 * k:128 * (k + 1)],
                                identity=identb[0:64, 0:64])
        fusedT_psm = p1bpsum.tile([128, 5, NTOK], f32)
        for m in range(5):
            nc.tensor.matmul(out=fusedT_psm[:, m, :],
                             lhsT=mem_sb[:, 128 * m:128 * (m + 1)],
                             rhs=bsel_sb[:, :], start=True, stop=True)
        fusedT_sb = spool.tile([128, 10, NTOK], bf16)
        nc.scalar.copy(fusedT_sb[:, 0:5, :], fusedT_pse[:, :, :])
        nc.scalar.copy(fusedT_sb[:, 5:10, :], fusedT_psm[:, :, :])

        # ---------------- attention (batched over all tokens) ----------------
        attnw_sb = dmapool.tile([128, 10, HID], bf16)
        nc.sync.dma_start(out=attnw_sb[:, :, :],
                          in_=attnw_d[:, :, :].rearrange("k p o -> p k o"))
        attnb_sb = dmapool.tile([1, HID], bf16)
        nc.sync.dma_start(out=attnb_sb[:, :], in_=attnb_d[:, :])

        attn_ps = p1bpsum.tile([NTOK, HID], f32)
        for n in range(2):
            sl = slice(512 * n, min(HID, 512 * (n + 1)))
            for k in range(10):
                mm(out=attn_ps[:, sl], lhsT=fusedT_sb[:, k, :],
                   rhs=attnw_sb[:, k, sl], start=(k == 0), stop=False)
            mm(out=attn_ps[:, sl], lhsT=ones64[:, :],
               rhs=attnb_sb[:, sl], start=False, stop=True)
        # softmax over free dim, then context = softmax * memory
        nmx_sb = scpool.tile([NTOK, 1], f32)
        nc.vector.reduce_max(out=nmx_sb[:, :], in_=attn_ps[:, :], axis=AX.X,
                             negate=True)
        ex_sb = scpool.tile([NTOK, HID], f32)
        ssum_sb = scpool.tile([NTOK, 1], f32)
        nc.scalar.activation(ex_sb[:, :], attn_ps[:, :], AF.Exp,
                             bias=nmx_sb[:, 0:1], accum_out=ssum_sb[:, 0:1])
        rcp_sb = scpool.tile([NTOK, 1], f32)
        nc.vector.reciprocal(rcp_sb[:, :], ssum_sb[:, :])
        ctx_sb = scpool.tile([NTOK, HID], bf16)
        nc.vector.tensor_scalar_mul(ctx_sb[:, :], ex_sb[:, :], rcp_sb[:, 0:1])
        nc.vector.tensor_tensor(out=ctx_sb[:, :], in0=ctx_sb[:, :], in1=mexp_sb[:, :],
                                op=ALU.mult)
        ctxT_ps = p1bpsum.tile([128, 5, NTOK], bf16)
        for k in range(5):
            nc.tensor.transpose(out=ctxT_ps[:, k, :],
                                in_=ctx_sb[:, 128 * k:128 * (k + 1)],
                                identity=identb[0:64, 0:64])
        ctxT_sb = spool.tile([128, 5, NTOK], bf16)
        nc.scalar.copy(ctxT_sb[:, :, :], ctxT_ps[:, :, :])
        p1bpsum.release()
        scpool.release()

        # ---------------- gates precompute: xin @ w_ih.T + (b_ih+b_hh) ----------------
        whh_sb = dmapool.tile([128, 5, 4 * HID], bf16)
        nc.sync.dma_start(out=whh_sb[:, :, :],
                          in_=whh_d[:, :, :].rearrange("k p o -> p k o"))

        p2psum = tc.alloc_tile_pool(name="p2ps", bufs=1, space="PSUM")
        P_ps = p2psum.tile([NTOK, 4 * HID], f32)
        bgate_sb = dmapool.tile([1, 4 * HID], bf16, tag="bgate", bufs=1)
        nc.sync.dma_start(out=bgate_sb[:, :], in_=bgate_d[:, :])
        for k in range(10):
            wih_k = dmapool.tile([128, 4 * HID], bf16, tag="wihk", bufs=3)
            nc.sync.dma_start(out=wih_k[:, :], in_=wih_d[k, :, :])
            lhsT = fusedT_sb[:, k, :] if k < 5 else ctxT_sb[:, k - 5, :]
            for n in range(5):
                sl = slice(512 * n, 512 * (n + 1))
                mm(out=P_ps[:, sl], lhsT=lhsT, rhs=wih_k[:, sl],
                   start=(k == 0), stop=False)
        for n in range(5):
            sl = slice(512 * n, 512 * (n + 1))
            mm(out=P_ps[:, sl], lhsT=ones64[:, :], rhs=bgate_sb[:, sl],
               start=False, stop=True)
        precomp_sb = spool.tile([NTOK, 4 * HID], bf16)
        nc.scalar.copy(precomp_sb[:, :], P_ps[:, :])
        p2psum.release()

        if upto == "pre":
            raise _PhaseExit(tc)
        # ---------------- LSTM recurrence ----------------
        outsT_sb = spool.tile([128, 5, NTOK], bf16)   # h.T for every step
        c_sb = spool.tile([BL, HID], f32)
        sig_sb = spool.tile([BL, 3 * HID], f32)
        g_sb = spool.tile([BL, HID], f32)
        ig_sb = spool.tile([BL, HID], f32)
        thc_sb = spool.tile([BL, HID], f32)
        h_sb = spool.tile([BL, HID], bf16)

        # FC weight stream: allocate + DMA before the LSTM so transfers overlap it
        lpsum = tc.alloc_tile_pool(name="lstm_ps", bufs=1, space="PSUM")
        CH = 1000
        fws = []
        for j in range(VOCAB // CH):
            fw = dmapool.tile([128, 5, CH], bf16, tag="fw", bufs=3)
            nc.sync.dma_start(out=fw[:, :, :],
                              in_=fcw_d[:, :, CH * j:CH * (j + 1)].rearrange(
                                  "k p o -> p k o"))
            fcb_sb = dmapool.tile([1, CH], bf16, tag="fcb", bufs=2)
            nc.sync.dma_start(out=fcb_sb[:, :], in_=fcb_d[:, CH * j:CH * (j + 1)])
            fws.append((fw, fcb_sb))
        for t in range(T):
            pstage = dmapool.tile([BL, 4 * HID], bf16, tag="pstage", bufs=2)
            nc.sync.dma_start(out=pstage[:, :], in_=precomp_sb[2 * t:2 * t + 2, :])
            gates_ps = lpsum.tile([BL, 4 * HID], f32, tag="gates")
            for n in range(5):
                sl = slice(512 * n, 512 * (n + 1))
                last = (t == 0)
                mm(out=gates_ps[:, sl], lhsT=identb[0:2, 0:2],
                   rhs=pstage[:, sl],
                   start=True, stop=last)
                if t > 0:
                    for k in range(5):
                        mm(
                            out=gates_ps[:, sl],
                            lhsT=outsT_sb[:, k, 2 * (t - 1):2 * t],
                            rhs=whh_sb[:, k, sl],
                            start=False, stop=(k == 4),
                        )
            # nonlinearities: [i,f,o] sigmoid, [g] tanh (host permuted gate order)
            nc.scalar.activation(sig_sb[:, :], gates_ps[:, 0:3 * HID], AF.Sigmoid)
            nc.scalar.activation(g_sb[:, :], gates_ps[:, 3 * HID:4 * HID], AF.Tanh)
            nc.vector.tensor_tensor(out=ig_sb[:, :], in0=sig_sb[:, 0:HID],
                                    in1=g_sb[:, :], op=ALU.mult)
            if t > 0:
                nc.vector.tensor_tensor(out=c_sb[:, :], in0=sig_sb[:, HID:2 * HID],
                                        in1=c_sb[:, :], op=ALU.mult)
                nc.vector.tensor_tensor(out=c_sb[:, :], in0=c_sb[:, :],
                                        in1=ig_sb[:, :], op=ALU.add)
            else:
                nc.vector.tensor_copy(out=c_sb[:, :], in_=ig_sb[:, :])
            nc.scalar.activation(thc_sb[:, :], c_sb[:, :], AF.Tanh)
            nc.vector.tensor_tensor(out=h_sb[:, :], in0=sig_sb[:, 2 * HID:3 * HID],
                                    in1=thc_sb[:, :], op=ALU.mult)
            hT_ps = lpsum.tile([128, 5, BL], bf16, tag="hT")
            for k in range(5):
                nc.tensor.transpose(out=hT_ps[:, k, :],
                                    in_=h_sb[:, 128 * k:128 * (k + 1)],
                                    identity=identb[0:2, 0:2])
            nc.scalar.copy(outsT_sb[:, :, 2 * t:2 * t + 2], hT_ps[:, :, :])
        lpsum.release()

        if upto == "lstm":
            raise _PhaseExit(tc)
        # ---------------- FC to vocab: logits = outs @ fc_w.T + fc_b ----------------
        fpsum = tc.alloc_tile_pool(name="fc_ps", bufs=4, space="PSUM")
        for j in range(VOCAB // CH):
            fw, fcb_sb = fws[j]
            for s in range(CH // 500):
                ps = fpsum.tile([NTOK, 500], f32, tag="ps")
                for k in range(5):
                    mm(out=ps[:, :], lhsT=outsT_sb[:, k, :],
                       rhs=fw[:, k, 500 * s:500 * (s + 1)],
                       start=(k == 0), stop=False)
                mm(out=ps[:, :], lhsT=ones64[:, :],
                   rhs=fcb_sb[:, 500 * s:500 * (s + 1)],
                   start=False, stop=True)
                lo = spool.tile([NTOK, 500], f32, tag="lo", bufs=4)
                nc.scalar.copy(lo[:, :], ps[:, :])
                nc.sync.dma_start(
                    out=logits_d[:, :, CH * j + 500 * s:CH * j + 500 * (s + 1)]
                        .rearrange("b t v -> t b v"),
                    in_=lo[:, :],
                )
        fpsum.release()
        spool.release()
        dmapool.release()
        cpool.release()
    except _PhaseExit:
        pass

    nc.finalize()
    return nc


def _prep_shared(inputs):
    """Host-side weight layout prep (shared across cores)."""
    import ml_dtypes
    bf = ml_dtypes.bfloat16
    f = np.float32
    perm = _gate_perm()
    w1 = inputs["cw1"].astype(f)
    w1b = w1.transpose(2, 3, 1, 0).reshape(27, 64).copy()
    cb1t = inputs["cb1"].astype(f).reshape(64, 1).copy()
    cb2t = inputs["cb2"].astype(f).reshape(128, 1).copy()
    w2t9 = inputs["cw2"].astype(f).transpose(2, 3, 1, 0).reshape(9, 64, 128)
    w3t9 = inputs["cw3"].astype(f).transpose(2, 3, 1, 0).reshape(9, 128, 256)
    w4t9 = inputs["cw4"].astype(f).transpose(2, 3, 1, 0).reshape(9, 2, 128, 512)
    cb3t = inputs["cb3"].astype(f).reshape(2, 128).T.copy()
    cb4t = inputs["cb4"].astype(f).reshape(4, 128).T.copy()
    encwt = inputs["enc_w"].astype(f).T.reshape(4, 128, HID).copy()
    encbt = inputs["enc_b"].astype(f).reshape(5, 128).T.copy()
    attnwt = inputs["attn_w"].astype(f).T.reshape(10, 128, HID).copy()
    attnb = inputs["attn_b"].astype(f)[None, :]
    wih = inputs["w_ih"].astype(f)[perm]
    whh = inputs["w_hh"].astype(f)[perm]
    wiht = wih.T.reshape(10, 128, 4 * HID).copy()
    whht = whh.T.reshape(5, 128, 4 * HID).copy()
    bgate = (inputs["b_ih"].astype(f) + inputs["b_hh"].astype(f))[perm][None, :]
    fcwt = inputs["fc_w"].astype(f).T.reshape(5, 128, VOCAB).copy()
    fcb = inputs["fc_b"].astype(f)[None, :]
    bsel = np.zeros((BL, NTOK), f)
    for p in range(NTOK):
        bsel[p % BL, p] = 1.0
    return dict(w1b=w1b.astype(bf), cb1t=cb1t, cb2t=cb2t,
                w2t9=w2t9.astype(bf), w3t9=w3t9.astype(bf), w4t9=w4t9.astype(bf),
                cb3t=cb3t, cb4t=cb4t, encwt=encwt, encbt=encbt,
                attnwt=attnwt.astype(bf), attnb=attnb.astype(bf),
                wiht=wiht.astype(bf), whht=whht.astype(bf), bgate=bgate.astype(bf),
                fcwt=fcwt.astype(bf), fcb=fcb.astype(bf), bsel=bsel,
                emb=inputs["emb"].astype(f).astype(bf))


def make_in_maps(inputs):
    """Build the per-core input maps (host-side im2col + weight prep)."""
    shared = _prep_shared(inputs)
    images = np.asarray(inputs["images"], np.float32)
    captions = np.asarray(inputs["captions"])

    import ml_dtypes
    imgp = np.zeros((16, 3, 226, 226), np.float32)
    imgp[:, :, 1:225, 1:225] = images
    s = imgp.strides
    win = np.lib.stride_tricks.as_strided(
        imgp, shape=(16, 3, 3, 3, 224, 224),
        strides=(s[0], s[1], s[2], s[3], s[2], s[3]))
    # rows (ky, kx, c) to match w1 layout
    imcol = win.transpose(0, 2, 3, 1, 4, 5).reshape(16, 27, 224 * 224)
    imgp = imcol.astype(ml_dtypes.bfloat16)
    in_maps = []
    for c in range(NCORES):
        caps = captions[BL * c:BL * (c + 1)].astype(np.int64).T.reshape(NTOK, 1)
        m = dict(shared)
        m["img"] = imgp[BL * c:BL * (c + 1)].copy()
        m["caps"] = caps.astype(np.int32)
        in_maps.append(m)
    return in_maps


def kernel(**inputs):
    from concourse.bass_utils import run_bass_kernel_spmd

    if "nc" not in _NC_CACHE:
        _NC_CACHE["nc"] = build_bass()
    nc = _NC_CACHE["nc"]

    in_maps = make_in_maps(inputs)
    res = run_bass_kernel_spmd(nc, in_maps, list(range(NCORES)))
    out = np.concatenate([res.results[c]["logits"] for c in range(NCORES)], axis=0)
    return out



# revision 9
# speedup vs baseline: 1.4537x; 1.4537x over previous
"""Trainium2 Bass kernel for CNN-encoder + attention-LSTM captioner + vocab FC.

Sharding: pure data-parallel over batch (16 images -> 8 cores x 2 images).
All weights replicated; no collectives. Host slices inputs / concatenates outputs.

Layout conventions (per core, B=2 local images, T=32 steps):
  - tokens are indexed p = t*2 + b  (t-major) so each LSTM step reads a
    contiguous partition pair from the batched precompute.
  - recurrent state h is kept transposed ([HID, 2] chunks) in outsT so it can
    feed the next step's matmul lhsT directly and the final FC lhsT.
"""

import os
import numpy as np

os.environ.setdefault("MYCRO_LOCAL_CACHE", "1")

HID = 640
VOCAB = 10000
T = 32
BL = 2            # local batch per core
NTOK = T * BL     # 64
NCORES = 8

F32 = None  # set lazily (mybir.dt.float32)


class _PhaseExit(Exception):
    def __init__(self, tc):
        self.tc = tc

_NC_CACHE = {}


def _gate_perm():
    # reference gate order [i, f, g, o] -> kernel order [i, f, o, g]
    return np.concatenate([
        np.arange(0, 1280),          # i, f
        np.arange(1920, 2560),       # o
        np.arange(1280, 1920),       # g
    ])


def build_bass(upto=None):
    import os
    upto = upto or os.environ.get("KERNEL_UPTO", "all")
    import concourse.bass as bass
    from concourse import bacc
    import concourse.tile_sem_assignment as tsa
    # Cap HWDGE sem lanes so pool-transition fan-ins stay under the
    # per-instruction sync-wait slot limits in walrus codegen.
    tsa.NUM_HWDGE_SEMS = 4
    import concourse.mybir as mybir
    import concourse.tile as tile
    from concourse.masks import make_identity

    f32 = mybir.dt.float32
    i32 = mybir.dt.int32
    AF = mybir.ActivationFunctionType
    ALU = mybir.AluOpType
    AX = mybir.AxisListType

    nc = bacc.Bacc(None)
    bf16 = mybir.dt.bfloat16

    def mm(out, lhsT, rhs, **kw):
        nc.tensor.matmul(out=out, lhsT=lhsT, rhs=rhs, **kw)

    # ---------------- DRAM parameters ----------------
    img_d = nc.declare_dram_parameter("img", [BL, 27, 224 * 224], bf16, isOutput=False)
    caps_d = nc.declare_dram_parameter("caps", [NTOK, 1], i32, isOutput=False)
    w1b_d = nc.declare_dram_parameter("w1b", [27, 64], bf16, isOutput=False)
    cb1_d = nc.declare_dram_parameter("cb1t", [64, 1], f32, isOutput=False)
    cb2_d = nc.declare_dram_parameter("cb2t", [128, 1], f32, isOutput=False)
    w2t9_d = nc.declare_dram_parameter("w2t9", [9, 64, 128], bf16, isOutput=False)
    w3t9_d = nc.declare_dram_parameter("w3t9", [9, 128, 256], bf16, isOutput=False)
    w4t9_d = nc.declare_dram_parameter("w4t9", [9, 2, 128, 512], bf16, isOutput=False)
    cb3_d = nc.declare_dram_parameter("cb3t", [128, 2], f32, isOutput=False)
    cb4_d = nc.declare_dram_parameter("cb4t", [128, 4], f32, isOutput=False)
    encw_d = nc.declare_dram_parameter("encwt", [4, 128, HID], f32, isOutput=False)
    encb_d = nc.declare_dram_parameter("encbt", [128, 5], f32, isOutput=False)
    emb_d = nc.declare_dram_parameter("emb", [VOCAB, HID], bf16, isOutput=False)
    attnw_d = nc.declare_dram_parameter("attnwt", [10, 128, HID], bf16, isOutput=False)
    attnb_d = nc.declare_dram_parameter("attnb", [1, HID], bf16, isOutput=False)
    wih_d = nc.declare_dram_parameter("wiht", [10, 128, 4 * HID], bf16, isOutput=False)
    whh_d = nc.declare_dram_parameter("whht", [5, 128, 4 * HID], bf16, isOutput=False)
    bgate_d = nc.declare_dram_parameter("bgate", [1, 4 * HID], bf16, isOutput=False)
    fcw_d = nc.declare_dram_parameter("fcwt", [5, 128, VOCAB], bf16, isOutput=False)
    fcb_d = nc.declare_dram_parameter("fcb", [1, VOCAB], bf16, isOutput=False)
    bsel_d = nc.declare_dram_parameter("bsel", [BL, NTOK], f32, isOutput=False)
    logits_d = nc.declare_dram_parameter("logits", [BL, T, VOCAB], f32, isOutput=True)

    try:
      with tile.TileContext(nc) as tc:
        # ---------------- persistent constants ----------------
        cpool = tc.alloc_tile_pool(name="const", bufs=1)
        # pool for all DMA-written tiles: never released mid-kernel so that
        # SBUF zone reuse never makes compute ops wait on DMA queue sems
        dmapool = tc.alloc_tile_pool(name="dmat", bufs=1)
        ident = cpool.tile([128, 128], f32)
        make_identity(nc, ident[:, :])
        identb = cpool.tile([128, 128], bf16)
        make_identity(nc, identb[:, :])
        ones64 = cpool.tile([1, 64], bf16)
        nc.gpsimd.memset(ones64[:, :], 1.0)
        bsel_sb = dmapool.tile([BL, NTOK], f32)
        nc.sync.dma_start(out=bsel_sb[:, :], in_=bsel_d[:, :])
        feat_sb = cpool.tile([128, 4, BL], f32)   # feat.T, K-chunked [128,4] per img

        w1b_sb = dmapool.tile([27, 64], bf16)
        nc.sync.dma_start(out=w1b_sb[:, :], in_=w1b_d[:, :])
        cb1_sb = dmapool.tile([64, 1], f32)
        nc.sync.dma_start(out=cb1_sb[:, :], in_=cb1_d[:, :])
        cb2_sb = dmapool.tile([128, 1], f32)
        nc.sync.dma_start(out=cb2_sb[:, :], in_=cb2_d[:, :])
        w2_sb = dmapool.tile([64, 9, 128], bf16)
        nc.sync.dma_start(out=w2_sb[:, :, :], in_=w2t9_d[:, :, :].rearrange("t p o -> p t o"))
        w3_sb = dmapool.tile([128, 9, 256], bf16)
        nc.sync.dma_start(out=w3_sb[:, :, :], in_=w3t9_d[:, :, :].rearrange("t p o -> p t o"))
        cb3_sb = dmapool.tile([128, 2], f32)
        nc.sync.dma_start(out=cb3_sb[:, :], in_=cb3_d[:, :])
        cb4_sb = dmapool.tile([128, 4], f32)
        nc.sync.dma_start(out=cb4_sb[:, :], in_=cb4_d[:, :])

        # ---------------- conv tower, per image ----------------
        # DMA tiles only needed during the conv phase live in their own pool so
        # their SBUF is reclaimed for the wih/fcw prefetch afterwards
        convdma = tc.alloc_tile_pool(name="convdma", bufs=1)
        for im in range(BL):
            ipool = tc.alloc_tile_pool(name=f"img{im}", bufs=1)
            # padded pool1 output (input of conv2)
            x2_pad = ipool.tile([64, 114, 114], bf16)
            nc.vector.memset(x2_pad[:, :, :], 0.0)

            # ---- conv1 (3->64) via im2col K=27, flat reads from padded DRAM ----
            c1pool = tc.alloc_tile_pool(name=f"c1_{im}", bufs=2)
            c1psum = tc.alloc_tile_pool(name=f"c1p_{im}", bufs=3, space="PSUM")
            R = 16
            for ch in range(224 // R):
                Y = R * ch
                rh = convdma.tile([27, R * 224], bf16, tag="rh", bufs=2)
                nc.sync.dma_start(out=rh[:, :],
                                  in_=img_d[im, :, Y * 224:(Y + R) * 224])
                rhv = rh.rearrange("p (r x) -> p r x", x=224)
                # 4 psum tiles of 4 rows (2 subtiles x 2 rows)
                for q in range(4):
                    ps = c1psum.tile([64, 2, 448], f32, padded_shape=[64, 2, 512], tag="ps")
                    for s in range(2):
                        r0 = q * 4 + s * 2
                        mm(
                            out=ps[:, s, :],
                            lhsT=w1b_sb[:, :],
                            rhs=rhv[:, r0:r0 + 2, :],
                            start=True, stop=True,
                        )
                    a1 = c1pool.tile([64, 4, 224], bf16, tag="a1")
                    nc.scalar.activation(
                        a1.rearrange("p (a b) x -> p a b x", b=2),
                        ps.rearrange("p a (b x) -> p a b x", b=2),
                        AF.Relu, bias=cb1_sb[:, 0:1])
                    t1 = c1pool.tile([64, 4, 112], bf16, tag="t1")
                    nc.vector.tensor_tensor(
                        out=t1[:, :, :],
                        in0=a1[:, :, 0:224:2], in1=a1[:, :, 1:224:2],
                        op=ALU.max,
                    )
                    # pool rows: out rows (16ch+4q)/2 .. +2
                    oy = (R * ch + 4 * q) // 2
                    nc.vector.tensor_tensor(
                        out=x2_pad[:, oy + 1:oy + 3, 1:113],
                        in0=t1[:, 0:4:2, :], in1=t1[:, 1:4:2, :],
                        op=ALU.max,
                    )
            c1psum.release()
            c1pool.release()

            # ---- conv2 (64->128) 9 taps K=64 (tap4 K=65 adds bias), pool -> x3_pad ----
            x3_pad = ipool.tile([128, 58, 58], bf16)
            nc.vector.memset(x3_pad[:, :, :], 0.0)
            c2psum = tc.alloc_tile_pool(name=f"c2p_{im}", bufs=3, space="PSUM")
            c2pool = tc.alloc_tile_pool(name=f"c2_{im}", bufs=2)
            for tl in range(14):  # 8 output rows per tile
                ps = c2psum.tile([128, 2, 448], f32, padded_shape=[128, 2, 512], tag="ps")
                for s in range(2):
                    y0 = tl * 8 + s * 4
                    for ky in range(3):
                        for kx in range(3):
                            tap = ky * 3 + kx
                            mm(
                                out=ps[:, s, :], lhsT=w2_sb[:, tap, :],
                                rhs=x2_pad[:, y0 + ky:y0 + ky + 4, kx:kx + 112],
                                start=(tap == 0), stop=(tap == 8),
                            )
                a2 = c2pool.tile([128, 8, 112], bf16, tag="a2")
                nc.scalar.activation(
                    a2.rearrange("p (a y) x -> p a y x", a=2),
                    ps.rearrange("p a (y x) -> p a y x", x=112),
                    AF.Relu, bias=cb2_sb[:, 0:1])
                t2 = c2pool.tile([128, 8, 56], bf16, tag="t2")
                nc.vector.tensor_tensor(
                    out=t2[:, :, :], in0=a2[:, :, 0:112:2], in1=a2[:, :, 1:112:2],
                    op=ALU.max,
                )
                t2b = c2pool.tile([128, 4, 56], bf16, tag="t2b")
                nc.vector.tensor_tensor(
                    out=t2b[:, :, :], in0=t2[:, 0:8:2, :], in1=t2[:, 1:8:2, :],
                    op=ALU.max,
                )
                nc.vector.tensor_copy(
                    out=x3_pad[:, tl * 4 + 1:tl * 4 + 5, 1:57],
                    in_=t2b[:, :, :],
                )
            c2psum.release()
            c2pool.release()

            # ---- conv3 (128->256) K=128, bias via ACT evict, pool -> x4_pad ----
            x4_pad = ipool.tile([128, 2, 30, 30], bf16)
            nc.vector.memset(x4_pad[:, :, :, :], 0.0)
            c3psum = tc.alloc_tile_pool(name=f"c3p_{im}", bufs=3, space="PSUM")
            c3pool = tc.alloc_tile_pool(name=f"c3_{im}", bufs=2)
            for m in range(2):
                for tl in range(7):  # 8 output rows per tile
                    ps = c3psum.tile([128, 448], f32, padded_shape=[128, 512], tag="ps")
                    y0 = tl * 8
                    for ky in range(3):
                        for kx in range(3):
                            tap = ky * 3 + kx
                            rhs = x3_pad[:, y0 + ky:y0 + ky + 8, kx:kx + 56]
                            mm(
                                out=ps[:, :],
                                lhsT=w3_sb[:, tap, 128 * m:128 * (m + 1)],
                                rhs=rhs,
                                start=(tap == 0), stop=(tap == 8),
                            )
                    a3 = c3pool.tile([128, 8, 56], bf16, tag="a3")
                    nc.scalar.activation(
                        a3[:, :, :],
                        ps.rearrange("p (y x) -> p y x", x=56),
                        AF.Relu, bias=cb3_sb[:, m:m + 1])
                    t3 = c3pool.tile([128, 8, 28], bf16, tag="t3")
                    nc.vector.tensor_tensor(
                        out=t3[:, :, :], in0=a3[:, :, 0:56:2], in1=a3[:, :, 1:56:2],
                        op=ALU.max,
                    )
                    nc.vector.tensor_tensor(
                        out=x4_pad[:, m, tl * 4 + 1:tl * 4 + 5, 1:29],
                        in0=t3[:, 0:8:2, :], in1=t3[:, 1:8:2, :],
                        op=ALU.max,
                    )
            c3psum.release()
            c3pool.release()

            # ---- conv4 (256->512) K=256 (2 chunks), no pool; mean via accum_out ----
            c4psum = tc.alloc_tile_pool(name=f"c4p_{im}", bufs=3, space="PSUM")
            c4pool = tc.alloc_tile_pool(name=f"c4_{im}", bufs=2)
            msum = ipool.tile([128, 4, 2], f32)
            for m in range(4):
                w4m = convdma.tile([128, 2, 9, 128], bf16, tag="w4m", bufs=2)
                for k2 in range(2):
                    nc.sync.dma_start(
                        out=w4m[:, k2, :, :],
                        in_=w4t9_d[:, k2, :, 128 * m:128 * (m + 1)].rearrange(
                            "t p o -> p t o"),
                    )
                ps = c4psum.tile([128, 2, 392], f32, padded_shape=[128, 2, 512], tag="ps")
                for s in range(2):
                    y0 = s * 14
                    first = True
                    for ky in range(3):
                        for kx in range(3):
                            tap = ky * 3 + kx
                            for k2 in range(2):
                                rhs = x4_pad[:, k2, y0 + ky:y0 + ky + 14, kx:kx + 28]
                                mm(
                                    out=ps[:, s, :],
                                    lhsT=w4m[:, k2, tap, :],
                                    rhs=rhs,
                                    start=first, stop=(tap == 8 and k2 == 1),
                                )
                                first = False
                a4 = c4pool.tile([128, 2, 392], bf16, tag="a4")
                for s in range(2):
                    nc.scalar.activation(a4[:, s, :], ps[:, s, :], AF.Relu,
                                         bias=cb4_sb[:, m:m + 1],
                                         accum_out=msum[:, m, s:s + 1])
            c4psum.release()
            c4pool.release()
            # feat.T[:, m] = (msum[:,m,0] + msum[:,m,1]) / 784
            tmpf = ipool.tile([128, 4], f32)
            nc.vector.tensor_tensor(out=tmpf[:, :], in0=msum[:, :, 0], in1=msum[:, :, 1],
                                    op=ALU.add)
            nc.vector.tensor_scalar_mul(feat_sb[:, :, im], tmpf[:, :], 1.0 / 784.0)
            ipool.release()
        convdma.release()

        if upto == "conv":
            raise _PhaseExit(tc)

        # ---------------- encoder linear: memory.T = enc_w @ feat.T + enc_b ----------------
        spool = tc.alloc_tile_pool(name="seq", bufs=1)
        scpool = tc.alloc_tile_pool(name="scratch", bufs=1)
        encw_sb = dmapool.tile([128, 4, HID], f32)
        nc.sync.dma_start(out=encw_sb[:, :, :], in_=encw_d[:, :, :].rearrange("k p o -> p k o"))
        encb_sb = dmapool.tile([128, 5], f32)
        nc.sync.dma_start(out=encb_sb[:, :], in_=encb_d[:, :])

        p1psum = tc.alloc_tile_pool(name="p1ps", bufs=1, space="PSUM")
        memT_ps = p1psum.tile([128, 5, BL], f32)
        for m in range(5):
            for k in range(4):
                nc.tensor.matmul(
                    out=memT_ps[:, m, :],
                    lhsT=encw_sb[:, k, 128 * m:128 * (m + 1)],
                    rhs=feat_sb[:, k, :],
                    start=(k == 0), stop=(k == 3),
                )
        memT_sb = spool.tile([128, 5, BL], f32)
        for m in range(5):
            nc.vector.tensor_scalar_add(memT_sb[:, m, :], memT_ps[:, m, :],
                                        encb_sb[:, m:m + 1])
        # memory non-transposed [2, 640]
        mem_ps = p1psum.tile([BL, HID], f32)
        for m in range(5):
            nc.tensor.transpose(out=mem_ps[:, 128 * m:128 * (m + 1)],
                                in_=memT_sb[:, m, :], identity=ident[:, :])
        mem_sb = scpool.tile([BL, HID], f32)
        nc.scalar.copy(mem_sb[:, :], mem_ps[:, :])

        # memory broadcast to all tokens [64, 640] via bsel matmul
        mexp_ps = p1psum.tile([NTOK, HID], f32)
        for n in range(2):
            sl = slice(512 * n, min(HID, 512 * (n + 1)))
            nc.tensor.matmul(out=mexp_ps[:, sl], lhsT=bsel_sb[:, :], rhs=mem_sb[:, sl],
                             start=True, stop=True)
        mexp_sb = scpool.tile([NTOK, HID], f32)
        nc.scalar.copy(mexp_sb[:, :], mexp_ps[:, :])
        p1psum.release()
        p1bpsum = tc.alloc_tile_pool(name="p1bps", bufs=1, space="PSUM")

        # ---------------- embeddings gather + fusedT ----------------
        idx_sb = dmapool.tile([NTOK, 1], i32)
        nc.sync.dma_start(out=idx_sb[:, :], in_=caps_d[:, :])
        e_sb = dmapool.tile([NTOK, HID], bf16)
        nc.gpsimd.indirect_dma_start(
            out=e_sb[:, :], out_offset=None,
            in_=emb_d[:, :],
            in_offset=bass.IndirectOffsetOnAxis(ap=idx_sb[:, :1], axis=0),
        )
        # fusedT [128, 10, 64]: chunks 0-4 = e.T ; 5-9 = memory.T broadcast
        fusedT_pse = p1bpsum.tile([128, 5, NTOK], bf16)
        for k in range(5):
            nc.tensor.transpose(out=fusedT_pse[:, k, :],
                                in_=e_sb[:, 128 * k:128 * (k + 1)],
                                identity=identb[0:64, 0:64])
        fusedT_psm = p1bpsum.tile([128, 5, NTOK], f32)
        for m in range(5):
            nc.tensor.matmul(out=fusedT_psm[:, m, :],
                             lhsT=mem_sb[:, 128 * m:128 * (m + 1)],
                             rhs=bsel_sb[:, :], start=True, stop=True)
        fusedT_sb = spool.tile([128, 10, NTOK], bf16)
        nc.scalar.copy(fusedT_sb[:, 0:5, :], fusedT_pse[:, :, :])
        nc.scalar.copy(fusedT_sb[:, 5:10, :], fusedT_psm[:, :, :])

        # ---------------- attention (batched over all tokens) ----------------
        attnw_sb = dmapool.tile([128, 10, HID], bf16)
        nc.sync.dma_start(out=attnw_sb[:, :, :],
                          in_=attnw_d[:, :, :].rearrange("k p o -> p k o"))
        attnb_sb = dmapool.tile([1, HID], bf16)
        nc.sync.dma_start(out=attnb_sb[:, :], in_=attnb_d[:, :])

        attn_ps = p1bpsum.tile([NTOK, HID], f32)
        for n in range(2):
            sl = slice(512 * n, min(HID, 512 * (n + 1)))
            for k in range(10):
                mm(out=attn_ps[:, sl], lhsT=fusedT_sb[:, k, :],
                   rhs=attnw_sb[:, k, sl], start=(k == 0), stop=False)
            mm(out=attn_ps[:, sl], lhsT=ones64[:, :],
               rhs=attnb_sb[:, sl], start=False, stop=True)
        # softmax over free dim, then context = softmax * memory
        nmx_sb = scpool.tile([NTOK, 1], f32)
        nc.vector.reduce_max(out=nmx_sb[:, :], in_=attn_ps[:, :], axis=AX.X,
                             negate=True)
        ex_sb = scpool.tile([NTOK, HID], f32)
        ssum_sb = scpool.tile([NTOK, 1], f32)
        nc.scalar.activation(ex_sb[:, :], attn_ps[:, :], AF.Exp,
                             bias=nmx_sb[:, 0:1], accum_out=ssum_sb[:, 0:1])
        rcp_sb = scpool.tile([NTOK, 1], f32)
        nc.vector.reciprocal(rcp_sb[:, :], ssum_sb[:, :])
        ctx_sb = scpool.tile([NTOK, HID], bf16)
        nc.vector.tensor_scalar_mul(ctx_sb[:, :], ex_sb[:, :], rcp_sb[:, 0:1])
        nc.vector.tensor_tensor(out=ctx_sb[:, :], in0=ctx_sb[:, :], in1=mexp_sb[:, :],
                                op=ALU.mult)
        ctxT_ps = p1bpsum.tile([128, 5, NTOK], bf16)
        for k in range(5):
            nc.tensor.transpose(out=ctxT_ps[:, k, :],
                                in_=ctx_sb[:, 128 * k:128 * (k + 1)],
                                identity=identb[0:64, 0:64])
        ctxT_sb = spool.tile([128, 5, NTOK], bf16)
        nc.scalar.copy(ctxT_sb[:, :, :], ctxT_ps[:, :, :])
        p1bpsum.release()
        scpool.release()

        # ---------------- gates precompute: xin @ w_ih.T + (b_ih+b_hh) ----------------
        whh_sb = dmapool.tile([128, 5, 4 * HID], bf16)
        nc.sync.dma_start(out=whh_sb[:, :, :],
                          in_=whh_d[:, :, :].rearrange("k p o -> p k o"))

        wpool = tc.alloc_tile_pool(name="wpool", bufs=1)
        p2psum = tc.alloc_tile_pool(name="p2ps", bufs=1, space="PSUM")
        P_ps = p2psum.tile([NTOK, 4 * HID], f32)
        bgate_sb = wpool.tile([1, 4 * HID], bf16, tag="bgate", bufs=1)
        nc.sync.dma_start(out=bgate_sb[:, :], in_=bgate_d[:, :])
        # prefetch all w_ih chunks up front so the DMAs overlap the attention phase
        wih_tiles = []
        for k in range(10):
            wih_k = wpool.tile([128, 4 * HID], bf16, tag="wihk", bufs=10)
            nc.sync.dma_start(out=wih_k[:, :], in_=wih_d[k, :, :])
            wih_tiles.append(wih_k)
        for k in range(10):
            lhsT = fusedT_sb[:, k, :] if k < 5 else ctxT_sb[:, k - 5, :]
            for n in range(5):
                sl = slice(512 * n, 512 * (n + 1))
                mm(out=P_ps[:, sl], lhsT=lhsT, rhs=wih_tiles[k][:, sl],
                   start=(k == 0), stop=False)
        for n in range(5):
            sl = slice(512 * n, 512 * (n + 1))
            mm(out=P_ps[:, sl], lhsT=ones64[:, :], rhs=bgate_sb[:, sl],
               start=False, stop=True)
        precomp_sb = spool.tile([NTOK, 4 * HID], bf16)
        nc.scalar.copy(precomp_sb[:, :], P_ps[:, :])
        p2psum.release()

        # transpose P -> PTT [128, 20, NTOK] so the recurrence runs with gates on
        # partitions (batch on the tiny free dim)
        p3psum = tc.alloc_tile_pool(name="p3ps", bufs=1, space="PSUM")
        ptt_ps = p3psum.tile([128, 20, NTOK], bf16)
        for gc in range(20):
            nc.tensor.transpose(out=ptt_ps[:, gc, :],
                                in_=precomp_sb[:, 128 * gc:128 * (gc + 1)],
                                identity=identb[0:64, 0:64])
        ptt_sb = spool.tile([128, 20, NTOK], bf16)
        nc.scalar.copy(ptt_sb[:, :, :], ptt_ps[:, :, :])
        p3psum.release()

        if upto == "pre":
            raise _PhaseExit(tc)
        # ---------------- LSTM recurrence (transposed: gates on partitions) ----------------
        # chunk layout along partitions-of-gates: 0:5=i, 5:10=f, 10:15=o, 15:20=g
        outsT_sb = spool.tile([128, 5, NTOK], bf16)   # h.T for every step
        c_sb = spool.tile([128, 5, BL], f32)
        sig_sb = spool.tile([128, 15, BL], f32)
        g_sb = spool.tile([128, 5, BL], f32)
        ig_sb = spool.tile([128, 5, BL], f32)
        thc_sb = spool.tile([128, 5, BL], f32)

        # FC weight stream: allocate + DMA before the LSTM so transfers overlap it
        lpsum = tc.alloc_tile_pool(name="lstm_ps", bufs=1, space="PSUM")
        CH = 1000
        fws = []
        for j in range(VOCAB // CH):
            fw = wpool.tile([128, 5, CH], bf16, tag="fw", bufs=6)
            nc.sync.dma_start(out=fw[:, :, :],
                              in_=fcw_d[:, :, CH * j:CH * (j + 1)].rearrange(
                                  "k p o -> p k o"))
            fcb_sb = wpool.tile([1, CH], bf16, tag="fcb", bufs=2)
            nc.sync.dma_start(out=fcb_sb[:, :], in_=fcb_d[:, CH * j:CH * (j + 1)])
            fws.append((fw, fcb_sb))

        # two PSUM gate tiles, alternated across steps; warm each bank once with a
        # start=True matmul so later accumulate-only groups add onto DVE-preloaded
        # values (pending-zero flags cleared by the warmup writes)
        gps_ab = []
        for i in range(2):
            gps_i = lpsum.tile([128, 20, BL], f32, tag=f"g{i}", name=f"gps{i}")
            gps_ab.append(gps_i)
        for i in range(2):
            mm(out=gps_ab[i][:, :, :], lhsT=identb[:, :],
               rhs=ptt_sb[:, :, 0:BL], start=True, stop=True)
        for t in range(T):
            gps = gps_ab[t % 2]
            # preload precomputed gates on DVE (off the PE critical path)
            nc.vector.tensor_copy(out=gps[:, :, :], in_=ptt_sb[:, :, 2 * t:2 * t + 2])
            if t > 0:
                for gc in range(20):
                    for k in range(5):
                        mm(
                            out=gps[:, gc, :],
                            lhsT=whh_sb[:, k, 128 * gc:128 * (gc + 1)],
                            rhs=outsT_sb[:, k, 2 * (t - 1):2 * t],
                            start=False, stop=False, skip_group_check=True,
                        )
            # nonlinearities: [i,f,o] sigmoid, [g] tanh (host permuted gate order)
            nc.scalar.activation(sig_sb[:, :, :], gps[:, 0:15, :], AF.Sigmoid)
            nc.scalar.activation(g_sb[:, :, :], gps[:, 15:20, :], AF.Tanh)
            nc.vector.tensor_tensor(out=ig_sb[:, :, :], in0=sig_sb[:, 0:5, :],
                                    in1=g_sb[:, :, :], op=ALU.mult)
            if t > 0:
                nc.vector.tensor_tensor(out=c_sb[:, :, :], in0=sig_sb[:, 5:10, :],
                                        in1=c_sb[:, :, :], op=ALU.mult)
                nc.vector.tensor_tensor(out=c_sb[:, :, :], in0=c_sb[:, :, :],
                                        in1=ig_sb[:, :, :], op=ALU.add)
            else:
                nc.vector.tensor_copy(out=c_sb[:, :, :], in_=ig_sb[:, :, :])
            nc.scalar.activation(thc_sb[:, :, :], c_sb[:, :, :], AF.Tanh)
            nc.vector.tensor_tensor(out=outsT_sb[:, :, 2 * t:2 * t + 2],
                                    in0=sig_sb[:, 10:15, :],
                                    in1=thc_sb[:, :, :], op=ALU.mult)
        lpsum.release()

        if upto == "lstm":
            raise _PhaseExit(tc)
        # ---------------- FC to vocab: logits = outs @ fc_w.T + fc_b ----------------
        fpsum = tc.alloc_tile_pool(name="fc_ps", bufs=4, space="PSUM")
        for j in range(VOCAB // CH):
            fw, fcb_sb = fws[j]
            for s in range(CH // 500):
                ps = fpsum.tile([NTOK, 500], f32, tag="ps")
                for k in range(5):
                    mm(out=ps[:, :], lhsT=outsT_sb[:, k, :],
                       rhs=fw[:, k, 500 * s:500 * (s + 1)],
                       start=(k == 0), stop=False)
                mm(out=ps[:, :], lhsT=ones64[:, :],
                   rhs=fcb_sb[:, 500 * s:500 * (s + 1)],
                   start=False, stop=True)
                lo = spool.tile([NTOK, 500], f32, tag="lo", bufs=4)
                nc.scalar.copy(lo[:, :], ps[:, :])
                nc.sync.dma_start(
                    out=logits_d[:, :, CH * j + 500 * s:CH * j + 500 * (s + 1)]
                        .rearrange("b t v -> t b v"),
                    in_=lo[:, :],
                )
        fpsum.release()
        wpool.release()
        spool.release()
        dmapool.release()
        cpool.release()
    except _PhaseExit:
        pass

    nc.finalize()
    return nc


def _prep_shared(inputs):
    """Host-side weight layout prep (shared across cores)."""
    import ml_dtypes
    bf = ml_dtypes.bfloat16
    f = np.float32
    perm = _gate_perm()
    w1 = inputs["cw1"].astype(f)
    w1b = w1.transpose(2, 3, 1, 0).reshape(27, 64).copy()
    cb1t = inputs["cb1"].astype(f).reshape(64, 1).copy()
    cb2t = inputs["cb2"].astype(f).reshape(128, 1).copy()
    w2t9 = inputs["cw2"].astype(f).transpose(2, 3, 1, 0).reshape(9, 64, 128)
    w3t9 = inputs["cw3"].astype(f).transpose(2, 3, 1, 0).reshape(9, 128, 256)
    w4t9 = inputs["cw4"].astype(f).transpose(2, 3, 1, 0).reshape(9, 2, 128, 512)
    cb3t = inputs["cb3"].astype(f).reshape(2, 128).T.copy()
    cb4t = inputs["cb4"].astype(f).reshape(4, 128).T.copy()
    encwt = inputs["enc_w"].astype(f).T.reshape(4, 128, HID).copy()
    encbt = inputs["enc_b"].astype(f).reshape(5, 128).T.copy()
    attnwt = inputs["attn_w"].astype(f).T.reshape(10, 128, HID).copy()
    attnb = inputs["attn_b"].astype(f)[None, :]
    wih = inputs["w_ih"].astype(f)[perm]
    whh = inputs["w_hh"].astype(f)[perm]
    wiht = wih.T.reshape(10, 128, 4 * HID).copy()
    whht = whh.T.reshape(5, 128, 4 * HID).copy()
    bgate = (inputs["b_ih"].astype(f) + inputs["b_hh"].astype(f))[perm][None, :]
    fcwt = inputs["fc_w"].astype(f).T.reshape(5, 128, VOCAB).copy()
    fcb = inputs["fc_b"].astype(f)[None, :]
    bsel = np.zeros((BL, NTOK), f)
    for p in range(NTOK):
        bsel[p % BL, p] = 1.0
    return dict(w1b=w1b.astype(bf), cb1t=cb1t, cb2t=cb2t,
                w2t9=w2t9.astype(bf), w3t9=w3t9.astype(bf), w4t9=w4t9.astype(bf),
                cb3t=cb3t, cb4t=cb4t, encwt=encwt, encbt=encbt,
                attnwt=attnwt.astype(bf), attnb=attnb.astype(bf),
                wiht=wiht.astype(bf), whht=whht.astype(bf), bgate=bgate.astype(bf),
                fcwt=fcwt.astype(bf), fcb=fcb.astype(bf), bsel=bsel,
                emb=inputs["emb"].astype(f).astype(bf))


def make_in_maps(inputs):
    """Build the per-core input maps (host-side im2col + weight prep)."""
    shared = _prep_shared(inputs)
    images = np.asarray(inputs["images"], np.float32)
    captions = np.asarray(inputs["captions"])

    import ml_dtypes
    imgp = np.zeros((16, 3, 226, 226), np.float32)
    imgp[:, :, 1:225, 1:225] = images
    s = imgp.strides
    win = np.lib.stride_tricks.as_strided(
        imgp, shape=(16, 3, 3, 3, 224, 224),
        strides=(s[0], s[1], s[2], s[3], s[2], s[3]))
    # rows (ky, kx, c) to match w1 layout
    imcol = win.transpose(0, 2, 3, 1, 4, 5).reshape(16, 27, 224 * 224)
    imgp = imcol.astype(ml_dtypes.bfloat16)
    in_maps = []
    for c in range(NCORES):
        caps = captions[BL * c:BL * (c + 1)].astype(np.int64).T.reshape(NTOK, 1)
        m = dict(shared)
        m["img"] = imgp[BL * c:BL * (c + 1)].copy()
        m["caps"] = caps.astype(np.int32)
        in_maps.append(m)
    return in_maps


def kernel(**inputs):
    from concourse.bass_utils import run_bass_kernel_spmd

    if "nc" not in _NC_CACHE:
        _NC_CACHE["nc"] = build_bass()
    nc = _NC_CACHE["nc"]

    in_maps = make_in_maps(inputs)
    res = run_bass_kernel_spmd(nc, in_maps, list(range(NCORES)))
    out = np.concatenate([res.results[c]["logits"] for c in range(NCORES)], axis=0)
    return out



# revision 16
# speedup vs baseline: 1.8579x; 1.2780x over previous
"""Trainium2 Bass kernel for CNN-encoder + attention-LSTM captioner + vocab FC.

Sharding: pure data-parallel over batch (16 images -> 8 cores x 2 images).
All weights replicated; no collectives. Host slices inputs / concatenates outputs.

Layout conventions (per core, B=2 local images, T=32 steps):
  - tokens are indexed p = t*2 + b  (t-major) so each LSTM step reads a
    contiguous partition pair from the batched precompute.
  - recurrent state h is kept transposed ([HID, 2] chunks) in outsT so it can
    feed the next step's matmul lhsT directly and the final FC lhsT.
"""

import os
import numpy as np

os.environ.setdefault("MYCRO_LOCAL_CACHE", "1")

HID = 640
VOCAB = 10000
T = 32
BL = 2            # local batch per core
NTOK = T * BL     # 64
NCORES = 8

F32 = None  # set lazily (mybir.dt.float32)


class _PhaseExit(Exception):
    def __init__(self, tc):
        self.tc = tc

_NC_CACHE = {}


def _gate_perm():
    # reference gate order [i, f, g, o] -> kernel order [i, f, o, g]
    return np.concatenate([
        np.arange(0, 1280),          # i, f
        np.arange(1920, 2560),       # o
        np.arange(1280, 1920),       # g
    ])


def build_bass(upto=None):
    import os
    upto = upto or os.environ.get("KERNEL_UPTO", "all")
    import concourse.bass as bass
    from concourse import bacc
    import concourse.tile_sem_assignment as tsa
    # Cap HWDGE sem lanes so pool-transition fan-ins stay under the
    # per-instruction sync-wait slot limits in walrus codegen.
    tsa.NUM_HWDGE_SEMS = 4
    import concourse.mybir as mybir
    import concourse.tile as tile
    from concourse.masks import make_identity

    f32 = mybir.dt.float32
    i32 = mybir.dt.int32
    AF = mybir.ActivationFunctionType
    ALU = mybir.AluOpType
    AX = mybir.AxisListType

    nc = bacc.Bacc(None)
    bf16 = mybir.dt.bfloat16

    def mm(out, lhsT, rhs, **kw):
        nc.tensor.matmul(out=out, lhsT=lhsT, rhs=rhs, **kw)

    # ---------------- DRAM parameters ----------------
    img_d = nc.declare_dram_parameter("img", [BL, 36, 112 * 224], bf16, isOutput=False)
    caps_d = nc.declare_dram_parameter("caps", [NTOK, 1], i32, isOutput=False)
    w1p_d = nc.declare_dram_parameter("w1p", [36, 128], bf16, isOutput=False)
    cb1_d = nc.declare_dram_parameter("cb1t", [128, 1], f32, isOutput=False)
    cb2_d = nc.declare_dram_parameter("cb2t", [128, 1], f32, isOutput=False)
    w2t9_d = nc.declare_dram_parameter("w2t9", [9, 64, 128], bf16, isOutput=False)
    w2p_d = nc.declare_dram_parameter("w2p", [3, 128, 128], bf16, isOutput=False)
    w3t9_d = nc.declare_dram_parameter("w3t9", [9, 128, 256], bf16, isOutput=False)
    w4t9_d = nc.declare_dram_parameter("w4t9", [9, 2, 128, 512], bf16, isOutput=False)
    cb3_d = nc.declare_dram_parameter("cb3t", [128, 2], f32, isOutput=False)
    cb4_d = nc.declare_dram_parameter("cb4t", [128, 4], f32, isOutput=False)
    encw_d = nc.declare_dram_parameter("encwt", [4, 128, HID], f32, isOutput=False)
    encb_d = nc.declare_dram_parameter("encbt", [128, 5], f32, isOutput=False)
    emb_d = nc.declare_dram_parameter("emb", [VOCAB, HID], bf16, isOutput=False)
    attnw_d = nc.declare_dram_parameter("attnwt", [10, 128, HID], bf16, isOutput=False)
    attnb_d = nc.declare_dram_parameter("attnb", [1, HID], bf16, isOutput=False)
    wih_d = nc.declare_dram_parameter("wiht", [10, 128, 4 * HID], bf16, isOutput=False)
    whh_d = nc.declare_dram_parameter("whht", [5, 128, 4 * HID], bf16, isOutput=False)
    bgate_d = nc.declare_dram_parameter("bgate", [1, 4 * HID], bf16, isOutput=False)
    fcw_d = nc.declare_dram_parameter("fcwt", [5, 128, VOCAB], bf16, isOutput=False)
    fcb_d = nc.declare_dram_parameter("fcb", [1, VOCAB], bf16, isOutput=False)
    bsel_d = nc.declare_dram_parameter("bsel", [BL, NTOK], f32, isOutput=False)
    logits_d = nc.declare_dram_parameter("logits", [BL, T, VOCAB], f32, isOutput=True)

    try:
      with tile.TileContext(nc) as tc:
        # ---------------- persistent constants ----------------
        cpool = tc.alloc_tile_pool(name="const", bufs=1)
        # pool for all DMA-written tiles: never released mid-kernel so that
        # SBUF zone reuse never makes compute ops wait on DMA queue sems
        dmapool = tc.alloc_tile_pool(name="dmat", bufs=1)
        ident = cpool.tile([128, 128], f32)
        make_identity(nc, ident[:, :])
        identb = cpool.tile([128, 128], bf16)
        make_identity(nc, identb[:, :])
        ones64 = cpool.tile([1, 64], bf16)
        nc.gpsimd.memset(ones64[:, :], 1.0)
        bsel_sb = dmapool.tile([BL, NTOK], f32)
        nc.sync.dma_start(out=bsel_sb[:, :], in_=bsel_d[:, :])
        feat_sb = cpool.tile([128, 4, BL], f32)   # feat.T, K-chunked [128,4] per img

        w1p_sb = dmapool.tile([36, 128], bf16)
        nc.sync.dma_start(out=w1p_sb[:, :], in_=w1p_d[:, :])
        cb1p_sb = dmapool.tile([128, 1], f32)
        nc.sync.dma_start(out=cb1p_sb[:, :], in_=cb1_d[:, :])
        cb2_sb = dmapool.tile([128, 1], f32)
        nc.sync.dma_start(out=cb2_sb[:, :], in_=cb2_d[:, :])
        w2_sb = dmapool.tile([64, 9, 128], bf16)
        nc.sync.dma_start(out=w2_sb[:, :, :], in_=w2t9_d[:, :, :].rearrange("t p o -> p t o"))
        w2p_sb = dmapool.tile([128, 3, 128], bf16)
        nc.sync.dma_start(out=w2p_sb[:, :, :], in_=w2p_d[:, :, :].rearrange("t p o -> p t o"))
        w3_sb = dmapool.tile([128, 9, 256], bf16)
        nc.sync.dma_start(out=w3_sb[:, :, :], in_=w3t9_d[:, :, :].rearrange("t p o -> p t o"))
        cb3_sb = dmapool.tile([128, 2], f32)
        nc.sync.dma_start(out=cb3_sb[:, :], in_=cb3_d[:, :])
        cb4_sb = dmapool.tile([128, 4], f32)
        nc.sync.dma_start(out=cb4_sb[:, :], in_=cb4_d[:, :])

        # ---------------- conv tower, per image ----------------
        # DMA tiles only needed during the conv phase live in their own pool so
        # their SBUF is reclaimed for the wih/fcw prefetch afterwards
        convdma = tc.alloc_tile_pool(name="convdma", bufs=1)
        for im in range(BL):
            ipool = tc.alloc_tile_pool(name=f"img{im}", bufs=1)
            # pool1 output, duplicated across the partition dim for conv2's
            # ky-pair packing: partitions 0:64 hold x2[c, y], partitions 64:128
            # hold x2[c, y+1] (written by an SBUF->SBUF partition-shift DMA)
            x2_dup = ipool.tile([128, 114, 114], bf16)
            nc.vector.memset(x2_dup[:, :, :], 0.0)

            # ---- conv1 (3->64), row-pair packed: K=36, out partitions (j, ch) ----
            c1pool = tc.alloc_tile_pool(name=f"c1_{im}", bufs=2)
            c1psum = tc.alloc_tile_pool(name=f"c1p_{im}", bufs=3, space="PSUM")
            for st in range(14):          # strips of 8 row-pairs
                rh = convdma.tile([36, 8 * 224], bf16, tag="rh", bufs=2)
                nc.sync.dma_start(out=rh[:, :],
                                  in_=img_d[im, :, st * 1792:(st + 1) * 1792])
                rhv = rh.rearrange("p (r x) -> p r x", x=224)
                t1s = c1pool.tile([128, 8, 112], bf16, tag="t1s")
                for q in range(4):
                    ps = c1psum.tile([128, 448], f32, padded_shape=[128, 512], tag="ps")
                    mm(
                        out=ps[:, :],
                        lhsT=w1p_sb[:, :],
                        rhs=rhv[:, 2 * q:2 * q + 2, :],
                        start=True, stop=True,
                    )
                    a1 = c1pool.tile([128, 2, 224], bf16, tag="a1")
                    nc.scalar.activation(
                        a1[:, :, :],
                        ps.rearrange("p (r x) -> p r x", x=224),
                        AF.Relu, bias=cb1p_sb[:, 0:1])
                    nc.vector.tensor_tensor(
                        out=t1s[:, 2 * q:2 * q + 2, :],
                        in0=a1[:, :, 0:224:2], in1=a1[:, :, 1:224:2],
                        op=ALU.max,
                    )
                # pool across the row-pair (partition halves) via shift DMA
                t1b = c1pool.tile([64, 8, 112], bf16, tag="t1b")
                nc.sync.dma_start(out=t1b[:, :, :], in_=t1s[64:128, :, :])
                oy = st * 8
                nc.vector.tensor_tensor(
                    out=x2_dup[0:64, oy + 1:oy + 9, 1:113],
                    in0=t1s[0:64, :, :], in1=t1b[:, :, :],
                    op=ALU.max,
                )
                # mirror rows (shifted by one) into partitions 64:128 for conv2
                nc.sync.dma_start(out=x2_dup[64:128, oy:oy + 8, :],
                                  in_=x2_dup[0:64, oy + 1:oy + 9, :])
            c1psum.release()
            c1pool.release()

            # ---- conv2 (64->128): ky 0+1 packed via x2_dup (K=128) + ky=2 (K=64) ----
            x3_pad = ipool.tile([128, 58, 58], bf16)
            nc.vector.memset(x3_pad[:, :, :], 0.0)
            c2psum = tc.alloc_tile_pool(name=f"c2p_{im}", bufs=3, space="PSUM")
            c2pool = tc.alloc_tile_pool(name=f"c2_{im}", bufs=2)
            for tl in range(14):  # 8 output rows per tile
                ps = c2psum.tile([128, 2, 448], f32, padded_shape=[128, 2, 512], tag="ps")
                for s in range(2):
                    y0 = tl * 8 + s * 4
                    for kx in range(3):
                        mm(
                            out=ps[:, s, :], lhsT=w2p_sb[:, kx, :],
                            rhs=x2_dup[:, y0:y0 + 4, kx:kx + 112],
                            start=(kx == 0), stop=False,
                        )
                    for kx in range(3):
                        mm(
                            out=ps[:, s, :], lhsT=w2_sb[:, 6 + kx, :],
                            rhs=x2_dup[0:64, y0 + 2:y0 + 6, kx:kx + 112],
                            start=False, stop=(kx == 2),
                        )
                a2 = c2pool.tile([128, 8, 112], bf16, tag="a2")
                nc.scalar.activation(
                    a2.rearrange("p (a y) x -> p a y x", a=2),
                    ps.rearrange("p a (y x) -> p a y x", x=112),
                    AF.Relu, bias=cb2_sb[:, 0:1])
                t2 = c2pool.tile([128, 8, 56], bf16, tag="t2")
                nc.vector.tensor_tensor(
                    out=t2[:, :, :], in0=a2[:, :, 0:112:2], in1=a2[:, :, 1:112:2],
                    op=ALU.max,
                )
                t2b = c2pool.tile([128, 4, 56], bf16, tag="t2b")
                nc.vector.tensor_tensor(
                    out=t2b[:, :, :], in0=t2[:, 0:8:2, :], in1=t2[:, 1:8:2, :],
                    op=ALU.max,
                )
                nc.vector.tensor_copy(
                    out=x3_pad[:, tl * 4 + 1:tl * 4 + 5, 1:57],
                    in_=t2b[:, :, :],
                )
            c2psum.release()
            c2pool.release()

            # ---- conv3 (128->256) K=128, bias via ACT evict, pool -> x4_pad ----
            x4_pad = ipool.tile([128, 2, 30, 30], bf16)
            nc.vector.memset(x4_pad[:, :, :, :], 0.0)
            c3psum = tc.alloc_tile_pool(name=f"c3p_{im}", bufs=3, space="PSUM")
            c3pool = tc.alloc_tile_pool(name=f"c3_{im}", bufs=2)
            for m in range(2):
                for tl in range(7):  # 8 output rows per tile
                    ps = c3psum.tile([128, 448], f32, padded_shape=[128, 512], tag="ps")
                    y0 = tl * 8
                    for ky in range(3):
                        for kx in range(3):
                            tap = ky * 3 + kx
                            rhs = x3_pad[:, y0 + ky:y0 + ky + 8, kx:kx + 56]
                            mm(
                                out=ps[:, :],
                                lhsT=w3_sb[:, tap, 128 * m:128 * (m + 1)],
                                rhs=rhs,
                                start=(tap == 0), stop=(tap == 8),
                            )
                    a3 = c3pool.tile([128, 8, 56], bf16, tag="a3")
                    nc.scalar.activation(
                        a3[:, :, :],
                        ps.rearrange("p (y x) -> p y x", x=56),
                        AF.Relu, bias=cb3_sb[:, m:m + 1])
                    t3 = c3pool.tile([128, 8, 28], bf16, tag="t3")
                    nc.vector.tensor_tensor(
                        out=t3[:, :, :], in0=a3[:, :, 0:56:2], in1=a3[:, :, 1:56:2],
                        op=ALU.max,
                    )
                    nc.vector.tensor_tensor(
                        out=x4_pad[:, m, tl * 4 + 1:tl * 4 + 5, 1:29],
                        in0=t3[:, 0:8:2, :], in1=t3[:, 1:8:2, :],
                        op=ALU.max,
                    )
            c3psum.release()
            c3pool.release()

            # ---- conv4 (256->512) K=256 (2 chunks), no pool; mean via accum_out ----
            c4psum = tc.alloc_tile_pool(name=f"c4p_{im}", bufs=3, space="PSUM")
            c4pool = tc.alloc_tile_pool(name=f"c4_{im}", bufs=2)
            msum = ipool.tile([128, 4, 2], f32)
            for m in range(4):
                w4m = convdma.tile([128, 2, 9, 128], bf16, tag="w4m", bufs=2)
                for k2 in range(2):
                    nc.sync.dma_start(
                        out=w4m[:, k2, :, :],
                        in_=w4t9_d[:, k2, :, 128 * m:128 * (m + 1)].rearrange(
                            "t p o -> p t o"),
                    )
                ps = c4psum.tile([128, 2, 392], f32, padded_shape=[128, 2, 512], tag="ps")
                for s in range(2):
                    y0 = s * 14
                    first = True
                    for ky in range(3):
                        for kx in range(3):
                            tap = ky * 3 + kx
                            for k2 in range(2):
                                rhs = x4_pad[:, k2, y0 + ky:y0 + ky + 14, kx:kx + 28]
                                mm(
                                    out=ps[:, s, :],
                                    lhsT=w4m[:, k2, tap, :],
                                    rhs=rhs,
                                    start=first, stop=(tap == 8 and k2 == 1),
                                )
                                first = False
                a4 = c4pool.tile([128, 2, 392], bf16, tag="a4")
                for s in range(2):
                    nc.scalar.activation(a4[:, s, :], ps[:, s, :], AF.Relu,
                                         bias=cb4_sb[:, m:m + 1],
                                         accum_out=msum[:, m, s:s + 1])
            c4psum.release()
            c4pool.release()
            # feat.T[:, m] = (msum[:,m,0] + msum[:,m,1]) / 784
            tmpf = ipool.tile([128, 4], f32)
            nc.vector.tensor_tensor(out=tmpf[:, :], in0=msum[:, :, 0], in1=msum[:, :, 1],
                                    op=ALU.add)
            nc.vector.tensor_scalar_mul(feat_sb[:, :, im], tmpf[:, :], 1.0 / 784.0)
            ipool.release()
        convdma.release()

        if upto == "conv":
            raise _PhaseExit(tc)

        # ---------------- encoder linear: memory.T = enc_w @ feat.T + enc_b ----------------
        spool = tc.alloc_tile_pool(name="seq", bufs=1)
        scpool = tc.alloc_tile_pool(name="scratch", bufs=1)
        encw_sb = dmapool.tile([128, 4, HID], f32)
        nc.sync.dma_start(out=encw_sb[:, :, :], in_=encw_d[:, :, :].rearrange("k p o -> p k o"))
        encb_sb = dmapool.tile([128, 5], f32)
        nc.sync.dma_start(out=encb_sb[:, :], in_=encb_d[:, :])

        p1psum = tc.alloc_tile_pool(name="p1ps", bufs=1, space="PSUM")
        memT_ps = p1psum.tile([128, 5, BL], f32)
        for m in range(5):
            for k in range(4):
                nc.tensor.matmul(
                    out=memT_ps[:, m, :],
                    lhsT=encw_sb[:, k, 128 * m:128 * (m + 1)],
                    rhs=feat_sb[:, k, :],
                    start=(k == 0), stop=(k == 3),
                )
        memT_sb = spool.tile([128, 5, BL], f32)
        for m in range(5):
            nc.vector.tensor_scalar_add(memT_sb[:, m, :], memT_ps[:, m, :],
                                        encb_sb[:, m:m + 1])
        # memory non-transposed [2, 640]
        mem_ps = p1psum.tile([BL, HID], f32)
        for m in range(5):
            nc.tensor.transpose(out=mem_ps[:, 128 * m:128 * (m + 1)],
                                in_=memT_sb[:, m, :], identity=ident[:, :])
        mem_sb = scpool.tile([BL, HID], f32)
        nc.scalar.copy(mem_sb[:, :], mem_ps[:, :])

        # memory broadcast to all tokens [64, 640] via bsel matmul
        mexp_ps = p1psum.tile([NTOK, HID], f32)
        for n in range(2):
            sl = slice(512 * n, min(HID, 512 * (n + 1)))
            nc.tensor.matmul(out=mexp_ps[:, sl], lhsT=bsel_sb[:, :], rhs=mem_sb[:, sl],
                             start=True, stop=True)
        mexp_sb = scpool.tile([NTOK, HID], f32)
        nc.scalar.copy(mexp_sb[:, :], mexp_ps[:, :])
        p1psum.release()
        p1bpsum = tc.alloc_tile_pool(name="p1bps", bufs=1, space="PSUM")

        # ---------------- embeddings gather + fusedT ----------------
        idx_sb = dmapool.tile([NTOK, 1], i32)
        nc.sync.dma_start(out=idx_sb[:, :], in_=caps_d[:, :])
        e_sb = dmapool.tile([NTOK, HID], bf16)
        nc.gpsimd.indirect_dma_start(
            out=e_sb[:, :], out_offset=None,
            in_=emb_d[:, :],
            in_offset=bass.IndirectOffsetOnAxis(ap=idx_sb[:, :1], axis=0),
        )
        # fusedT [128, 10, 64]: chunks 0-4 = e.T ; 5-9 = memory.T broadcast
        fusedT_pse = p1bpsum.tile([128, 5, NTOK], bf16)
        for k in range(5):
            nc.tensor.transpose(out=fusedT_pse[:, k, :],
                                in_=e_sb[:, 128 * k:128 * (k + 1)],
                                identity=identb[0:64, 0:64])
        fusedT_psm = p1bpsum.tile([128, 5, NTOK], f32)
        for m in range(5):
            nc.tensor.matmul(out=fusedT_psm[:, m, :],
                             lhsT=mem_sb[:, 128 * m:128 * (m + 1)],
                             rhs=bsel_sb[:, :], start=True, stop=True)
        fusedT_sb = spool.tile([128, 10, NTOK], bf16)
        nc.scalar.copy(fusedT_sb[:, 0:5, :], fusedT_pse[:, :, :])
        nc.scalar.copy(fusedT_sb[:, 5:10, :], fusedT_psm[:, :, :])

        # ---------------- attention (batched over all tokens) ----------------
        attnw_sb = dmapool.tile([128, 10, HID], bf16)
        nc.sync.dma_start(out=attnw_sb[:, :, :],
                          in_=attnw_d[:, :, :].rearrange("k p o -> p k o"))
        attnb_sb = dmapool.tile([1, HID], bf16)
        nc.sync.dma_start(out=attnb_sb[:, :], in_=attnb_d[:, :])

        attn_ps = p1bpsum.tile([NTOK, HID], f32)
        for n in range(2):
            sl = slice(512 * n, min(HID, 512 * (n + 1)))
            for k in range(10):
                mm(out=attn_ps[:, sl], lhsT=fusedT_sb[:, k, :],
                   rhs=attnw_sb[:, k, sl], start=(k == 0), stop=False)
            mm(out=attn_ps[:, sl], lhsT=ones64[:, :],
               rhs=attnb_sb[:, sl], start=False, stop=True)
        # softmax over free dim, then context = softmax * memory
        nmx_sb = scpool.tile([NTOK, 1], f32)
        nc.vector.reduce_max(out=nmx_sb[:, :], in_=attn_ps[:, :], axis=AX.X,
                             negate=True)
        ex_sb = scpool.tile([NTOK, HID], f32)
        ssum_sb = scpool.tile([NTOK, 1], f32)
        nc.scalar.activation(ex_sb[:, :], attn_ps[:, :], AF.Exp,
                             bias=nmx_sb[:, 0:1], accum_out=ssum_sb[:, 0:1])
        rcp_sb = scpool.tile([NTOK, 1], f32)
        nc.vector.reciprocal(rcp_sb[:, :], ssum_sb[:, :])
        ctx_sb = scpool.tile([NTOK, HID], bf16)
        nc.vector.tensor_scalar_mul(ctx_sb[:, :], ex_sb[:, :], rcp_sb[:, 0:1])
        nc.vector.tensor_tensor(out=ctx_sb[:, :], in0=ctx_sb[:, :], in1=mexp_sb[:, :],
                                op=ALU.mult)
        ctxT_ps = p1bpsum.tile([128, 5, NTOK], bf16)
        for k in range(5):
            nc.tensor.transpose(out=ctxT_ps[:, k, :],
                                in_=ctx_sb[:, 128 * k:128 * (k + 1)],
                                identity=identb[0:64, 0:64])
        ctxT_sb = spool.tile([128, 5, NTOK], bf16)
        nc.scalar.copy(ctxT_sb[:, :, :], ctxT_ps[:, :, :])
        p1bpsum.release()
        scpool.release()

        # ---------------- gates precompute: xin @ w_ih.T + (b_ih+b_hh) ----------------
        whh_sb = dmapool.tile([128, 5, 4 * HID], bf16)
        nc.sync.dma_start(out=whh_sb[:, :, :],
                          in_=whh_d[:, :, :].rearrange("k p o -> p k o"))

        wihpool = tc.alloc_tile_pool(name="wihpool", bufs=1)
        p2psum = tc.alloc_tile_pool(name="p2ps", bufs=1, space="PSUM")
        P_ps = p2psum.tile([NTOK, 4 * HID], f32)
        bgate_sb = wihpool.tile([1, 4 * HID], bf16, tag="bgate", bufs=1)
        nc.sync.dma_start(out=bgate_sb[:, :], in_=bgate_d[:, :])
        # prefetch all w_ih chunks up front so the DMAs overlap the attention phase
        wih_tiles = []
        for k in range(10):
            wih_k = wihpool.tile([128, 4 * HID], bf16, tag="wihk", bufs=10)
            nc.sync.dma_start(out=wih_k[:, :], in_=wih_d[k, :, :])
            wih_tiles.append(wih_k)
        for k in range(10):
            lhsT = fusedT_sb[:, k, :] if k < 5 else ctxT_sb[:, k - 5, :]
            for n in range(5):
                sl = slice(512 * n, 512 * (n + 1))
                mm(out=P_ps[:, sl], lhsT=lhsT, rhs=wih_tiles[k][:, sl],
                   start=(k == 0), stop=False)
        for n in range(5):
            sl = slice(512 * n, 512 * (n + 1))
            mm(out=P_ps[:, sl], lhsT=ones64[:, :], rhs=bgate_sb[:, sl],
               start=False, stop=True)
        precomp_sb = spool.tile([NTOK, 4 * HID], bf16)
        nc.scalar.copy(precomp_sb[:, :], P_ps[:, :])
        p2psum.release()

        # transpose P -> PTT [128, 20, NTOK] so the recurrence runs with gates on
        # partitions (batch on the tiny free dim)
        p3psum = tc.alloc_tile_pool(name="p3ps", bufs=1, space="PSUM")
        ptt_ps = p3psum.tile([128, 20, NTOK], bf16)
        for gc in range(20):
            nc.tensor.transpose(out=ptt_ps[:, gc, :],
                                in_=precomp_sb[:, 128 * gc:128 * (gc + 1)],
                                identity=identb[0:64, 0:64])
        ptt_sb = spool.tile([128, 20, NTOK], bf16)
        nc.scalar.copy(ptt_sb[:, :, :], ptt_ps[:, :, :])
        p3psum.release()
        wihpool.release()
        # w_ih SBUF is reclaimed for the full fc_w prefetch
        fwpool = tc.alloc_tile_pool(name="fwpool", bufs=1)

        if upto == "pre":
            raise _PhaseExit(tc)
        # ---------------- LSTM recurrence (transposed: gates on partitions) ----------------
        # chunk layout along partitions-of-gates: 0:5=i, 5:10=f, 10:15=o, 15:20=g
        outsT_sb = spool.tile([128, 5, NTOK], bf16)   # h.T for every step
        c_sb = spool.tile([128, 5, BL], f32)
        sig_sb = spool.tile([128, 15, BL], f32)
        g_sb = spool.tile([128, 5, BL], f32)
        ig_sb = spool.tile([128, 5, BL], f32)
        thc_sb = spool.tile([128, 5, BL], f32)

        # FC weight stream: allocate + DMA before the LSTM so transfers overlap it
        lpsum = tc.alloc_tile_pool(name="lstm_ps", bufs=1, space="PSUM")
        CH = 1000
        fws = []
        for j in range(VOCAB // CH):
            fw = fwpool.tile([128, 5, CH], bf16, tag="fw", bufs=10)
            nc.sync.dma_start(out=fw[:, :, :],
                              in_=fcw_d[:, :, CH * j:CH * (j + 1)].rearrange(
                                  "k p o -> p k o"))
            fcb_sb = fwpool.tile([1, CH], bf16, tag="fcb", bufs=10)
            nc.sync.dma_start(out=fcb_sb[:, :], in_=fcb_d[:, CH * j:CH * (j + 1)])
            fws.append((fw, fcb_sb))

        # two PSUM gate tiles, alternated across steps; warm each bank once with a
        # start=True matmul so later accumulate-only groups add onto DVE-preloaded
        # values (pending-zero flags cleared by the warmup writes)
        gps_ab = []
        for i in range(2):
            gps_i = lpsum.tile([128, 20, BL], f32, tag=f"g{i}", name=f"gps{i}")
            gps_ab.append(gps_i)
        for i in range(2):
            mm(out=gps_ab[i][:, :, :], lhsT=identb[:, :],
               rhs=ptt_sb[:, :, 0:BL], start=True, stop=True)
        for t in range(T):
            gps = gps_ab[t % 2]
            # preload precomputed gates on DVE (off the PE critical path)
            nc.vector.tensor_copy(out=gps[:, :, :], in_=ptt_sb[:, :, 2 * t:2 * t + 2])
            if t > 0:
                for gc in range(20):
                    for k in range(5):
                        mm(
                            out=gps[:, gc, :],
                            lhsT=whh_sb[:, k, 128 * gc:128 * (gc + 1)],
                            rhs=outsT_sb[:, k, 2 * (t - 1):2 * t],
                            start=False, stop=False, skip_group_check=True,
                        )
            # nonlinearities: [i,f,o] sigmoid, [g] tanh (host permuted gate order)
            nc.scalar.activation(sig_sb[:, :, :], gps[:, 0:15, :], AF.Sigmoid)
            nc.scalar.activation(g_sb[:, :, :], gps[:, 15:20, :], AF.Tanh)
            nc.vector.tensor_tensor(out=ig_sb[:, :, :], in0=sig_sb[:, 0:5, :],
                                    in1=g_sb[:, :, :], op=ALU.mult)
            if t > 0:
                nc.vector.tensor_tensor(out=c_sb[:, :, :], in0=sig_sb[:, 5:10, :],
                                        in1=c_sb[:, :, :], op=ALU.mult)
                nc.vector.tensor_tensor(out=c_sb[:, :, :], in0=c_sb[:, :, :],
                                        in1=ig_sb[:, :, :], op=ALU.add)
            else:
                nc.vector.tensor_copy(out=c_sb[:, :, :], in_=ig_sb[:, :, :])
            nc.scalar.activation(thc_sb[:, :, :], c_sb[:, :, :], AF.Tanh)
            nc.vector.tensor_tensor(out=outsT_sb[:, :, 2 * t:2 * t + 2],
                                    in0=sig_sb[:, 10:15, :],
                                    in1=thc_sb[:, :, :], op=ALU.mult)
        lpsum.release()

        if upto == "lstm":
            raise _PhaseExit(tc)
        # ---------------- FC to vocab: logits = outs @ fc_w.T + fc_b ----------------
        fpsum = tc.alloc_tile_pool(name="fc_ps", bufs=4, space="PSUM")
        for j in range(VOCAB // CH):
            fw, fcb_sb = fws[j]
            for s in range(CH // 500):
                ps = fpsum.tile([NTOK, 500], f32, tag="ps")
                for k in range(5):
                    mm(out=ps[:, :], lhsT=outsT_sb[:, k, :],
                       rhs=fw[:, k, 500 * s:500 * (s + 1)],
                       start=(k == 0), stop=False)
                mm(out=ps[:, :], lhsT=ones64[:, :],
                   rhs=fcb_sb[:, 500 * s:500 * (s + 1)],
                   start=False, stop=True)
                lo = spool.tile([NTOK, 500], f32, tag="lo", bufs=4)
                nc.scalar.copy(lo[:, :], ps[:, :])
                nc.sync.dma_start(
                    out=logits_d[:, :, CH * j + 500 * s:CH * j + 500 * (s + 1)]
                        .rearrange("b t v -> t b v"),
                    in_=lo[:, :],
                )
        fpsum.release()
        fwpool.release()
        spool.release()
        dmapool.release()
        cpool.release()
    except _PhaseExit:
        pass

    nc.finalize()
    return nc


def _prep_shared(inputs):
    """Host-side weight layout prep (shared across cores)."""
    import ml_dtypes
    bf = ml_dtypes.bfloat16
    f = np.float32
    perm = _gate_perm()
    w1 = inputs["cw1"].astype(f)
    # row-pair packed conv1 weights: K rows (iy, kx, c), out cols (j, ch)
    # w1p[(iy,kx,c), j*64+ch] = w1[ch, c, iy-j, kx] for 0 <= iy-j <= 2
    w1p = np.zeros((4, 3, 3, 2, 64), f)
    for j in range(2):
        for iy in range(4):
            ky = iy - j
            if 0 <= ky <= 2:
                # w1 [ch, c, ky, kx] -> [kx, c, ch]
                w1p[iy, :, :, j, :] = w1[:, :, ky, :].transpose(2, 1, 0)
    w1p = w1p.reshape(36, 128).copy()
    cb1t = np.tile(inputs["cb1"].astype(f), 2).reshape(128, 1).copy()
    cb2t = inputs["cb2"].astype(f).reshape(128, 1).copy()
    w2t9 = inputs["cw2"].astype(f).transpose(2, 3, 1, 0).reshape(9, 64, 128)
    # ky-pair packed conv2 weights: w2p[kx, c, o] = w2 tap ky=0; [kx, 64+c, o] = ky=1
    w2p = np.stack([np.concatenate([w2t9[kx], w2t9[3 + kx]], axis=0)
                    for kx in range(3)])
    w3t9 = inputs["cw3"].astype(f).transpose(2, 3, 1, 0).reshape(9, 128, 256)
    w4t9 = inputs["cw4"].astype(f).transpose(2, 3, 1, 0).reshape(9, 2, 128, 512)
    cb3t = inputs["cb3"].astype(f).reshape(2, 128).T.copy()
    cb4t = inputs["cb4"].astype(f).reshape(4, 128).T.copy()
    encwt = inputs["enc_w"].astype(f).T.reshape(4, 128, HID).copy()
    encbt = inputs["enc_b"].astype(f).reshape(5, 128).T.copy()
    attnwt = inputs["attn_w"].astype(f).T.reshape(10, 128, HID).copy()
    attnb = inputs["attn_b"].astype(f)[None, :]
    wih = inputs["w_ih"].astype(f)[perm]
    whh = inputs["w_hh"].astype(f)[perm]
    wiht = wih.T.reshape(10, 128, 4 * HID).copy()
    whht = whh.T.reshape(5, 128, 4 * HID).copy()
    bgate = (inputs["b_ih"].astype(f) + inputs["b_hh"].astype(f))[perm][None, :]
    fcwt = inputs["fc_w"].astype(f).T.reshape(5, 128, VOCAB).copy()
    fcb = inputs["fc_b"].astype(f)[None, :]
    bsel = np.zeros((BL, NTOK), f)
    for p in range(NTOK):
        bsel[p % BL, p] = 1.0
    return dict(w1p=w1p.astype(bf), cb1t=cb1t, cb2t=cb2t,
                w2t9=w2t9.astype(bf), w2p=w2p.astype(bf),
                w3t9=w3t9.astype(bf), w4t9=w4t9.astype(bf),
                cb3t=cb3t, cb4t=cb4t, encwt=encwt, encbt=encbt,
                attnwt=attnwt.astype(bf), attnb=attnb.astype(bf),
                wiht=wiht.astype(bf), whht=whht.astype(bf), bgate=bgate.astype(bf),
                fcwt=fcwt.astype(bf), fcb=fcb.astype(bf), bsel=bsel,
                emb=inputs["emb"].astype(f).astype(bf))


def prep_images(images):
    """Row-pair packed im2col: [N, 36, 112*224] bf16, rows (iy, kx, c),
    cols (yb, x) with value imgpad[c, 2*yb+iy, x+kx]."""
    import ml_dtypes
    n = images.shape[0]
    imgp = np.zeros((n, 3, 226, 226), np.float32)
    imgp[:, :, 1:225, 1:225] = images
    s = imgp.strides
    win = np.lib.stride_tricks.as_strided(
        imgp, shape=(n, 3, 4, 3, 112, 224),
        strides=(s[0], s[1], s[2], s[3], 2 * s[2], s[3]))
    imcol = win.transpose(0, 2, 3, 1, 4, 5).reshape(n, 36, 112 * 224)
    return imcol.astype(ml_dtypes.bfloat16)


def make_in_maps(inputs):
    """Build the per-core input maps (host-side im2col + weight prep)."""
    shared = _prep_shared(inputs)
    images = np.asarray(inputs["images"], np.float32)
    captions = np.asarray(inputs["captions"])

    imgp = prep_images(images)
    in_maps = []
    for c in range(NCORES):
        caps = captions[BL * c:BL * (c + 1)].astype(np.int64).T.reshape(NTOK, 1)
        m = dict(shared)
        m["img"] = imgp[BL * c:BL * (c + 1)].copy()
        m["caps"] = caps.astype(np.int32)
        in_maps.append(m)
    return in_maps


def kernel(**inputs):
    from concourse.bass_utils import run_bass_kernel_spmd

    if "nc" not in _NC_CACHE:
        _NC_CACHE["nc"] = build_bass()
    nc = _NC_CACHE["nc"]

    in_maps = make_in_maps(inputs)
    res = run_bass_kernel_spmd(nc, in_maps, list(range(NCORES)))
    out = np.concatenate([res.results[c]["logits"] for c in range(NCORES)], axis=0)
    return out

